# revision 1
# baseline (speedup 1.0000x reference)
"""Multi-head self-attention (RoPE, causal) Bass kernel for 8 TRN2 NeuronCores.

Sharding: tensor-parallel over heads for QKV+attention (2 heads/core),
AllToAll, then token-parallel O-projection (512 tokens/core).

Layouts (per core):
  qT/kT/vT: [128 part = 2 heads x 64 dk, t]  (projection outputs, head-major)
  scoresT:  [128 part = k-tile, q free]      (softmax sum via ones-row matmul)
  v_sb:     [128 part = k-tile tokens, 130]  ([v_h0 | ones | v_h1 | ones])
  aoT:      [128 d, t]  attention output, normalized, pre-O-projection
  y:        [t, o] token-major final output

Causal masking: -1e9 mask matrices accumulated into the score PSUM via an
identity-stationary matmul (keeps masking on the PE, off the DVE/GpSimd).
"""

import numpy as np

B, S, D, H, DK = 2, 2048, 1024, 16, 64
NC = 8
THETA = 10000.0

_COMPILED = {}


def _build():
    import concourse.bass as bass
    import concourse.tile as tile
    from concourse import bacc, mybir

    f32 = mybir.dt.float32
    f32r = mybir.dt.float32r
    MUL = mybir.AluOpType.mult
    ADD = mybir.AluOpType.add
    EXP = mybir.ActivationFunctionType.Exp

    nc = bacc.Bacc(num_devices=NC)

    xt_d = nc.dram_tensor("xt", [B, D, S], f32r, kind="ExternalInput")
    wqt_d = nc.dram_tensor("wqt", [D, 128], f32r, kind="ExternalInput")
    wkt_d = nc.dram_tensor("wkt", [D, 128], f32r, kind="ExternalInput")
    wvt_d = nc.dram_tensor("wvt", [D, 128], f32r, kind="ExternalInput")
    wot_d = nc.dram_tensor("wot", [D, D], f32r, kind="ExternalInput")
    cost_d = nc.dram_tensor("cost", [128, S], f32, kind="ExternalInput")
    sinmt_d = nc.dram_tensor("sinmt", [128, S], f32, kind="ExternalInput")
    masks_d = nc.dram_tensor("masks", [2, 128, 256], f32r, kind="ExternalInput")
    ident_d = nc.dram_tensor("ident", [128, 128], f32, kind="ExternalInput")
    identr_d = nc.dram_tensor("identr", [128, 128], f32r, kind="ExternalInput")
    ones_d = nc.dram_tensor("ones", [128, 16], f32r, kind="ExternalInput")
    y_d = nc.dram_tensor("y", [B, S // NC, D], f32, kind="ExternalOutput")

    SWAP_MASK = [(i ^ 1) for i in range(32)]

    with tile.TileContext(nc) as tc:
        with (
            tc.tile_pool(name="const", bufs=1) as constp,
            tc.tile_pool(name="xtp", bufs=2) as xtp,
            tc.tile_pool(name="qk", bufs=1) as qkp,
            tc.tile_pool(name="vp", bufs=1) as vp,
            tc.tile_pool(name="attn", bufs=3) as attnp,
            tc.tile_pool(name="ao", bufs=1) as aop,
            tc.tile_pool(name="small", bufs=1) as smallp,
            tc.tile_pool(name="rbp", bufs=2) as rbp,
            tc.tile_pool(name="rtmp", bufs=2) as rtmp,
            tc.tile_pool(name="oproj", bufs=1) as op_,
            tc.tile_pool(name="yp", bufs=1) as yp,
            tc.tile_pool(name="ps", bufs=4, space="PSUM") as psp,
            tc.tile_pool(name="dram", bufs=1, space="DRAM") as dramp,
        ):
            # ---- constant tiles (loads emitted as late as their first use allows) ----
            cost = constp.tile([128, S], f32)
            sinmt = constp.tile([128, S], f32)
            masks = constp.tile([128, 2, 256], f32r)
            ident = constp.tile([128, 128], f32)
            identr = constp.tile([128, 128], f32r)
            ones_sb = constp.tile([128, 16], f32r)
            wq_sb = constp.tile([128, 8, 128], f32r)
            wk_sb = constp.tile([128, 8, 128], f32r)
            wv_sb = constp.tile([128, 8, 128], f32r)
            wo_sb = constp.tile([128, 8, D], f32r)

            # critical path: projection weights (sync queue, ahead of xt tiles)
            for dc in range(8):
                dsl = slice(dc * 128, (dc + 1) * 128)
                nc.sync.dma_start(wq_sb[:, dc, :], wqt_d[dsl, :])
                nc.sync.dma_start(wk_sb[:, dc, :], wkt_d[dsl, :])
                nc.sync.dma_start(wv_sb[:, dc, :], wvt_d[dsl, :])

            warm_in = dramp.tile([NC, 64], f32r, name="warm_in")
            warm_out = dramp.tile([NC, 64], f32r, name="warm_out")
            nc.gpsimd.collective_compute(
                "AllToAll",
                mybir.AluOpType.bypass,
                replica_groups=[list(range(NC))],
                ins=[warm_in.opt()],
                outs=[warm_out.opt()],
            )
            a2a_in = [dramp.tile([NC, 128, 256], f32r, name=f"a2ai{i}") for i in range(B)]
            a2a_out = [dramp.tile([NC, 128, 256], f32r, name=f"a2ao{i}") for i in range(B)]
            recip_dram = dramp.tile([B, 8, 512], f32)

            def o_projection(u):
                g = op_.tile([128, 8, 256], f32r, tag="g", name="g")
                for s in range(NC):
                    nc.sync.dma_start(g[:, s, :], a2a_out[u][s])
                y_sb = yp.tile([128, 2, D], f32, tag="y", name="y_sb")
                for ttt in range(2):
                    y_ps = psp.tile([128, 1024], f32, tag="ps", name="y_ps")
                    for os_ in range(2):
                        for dc in range(8):
                            nc.tensor.matmul(
                                y_ps[:, os_ * 512:(os_ + 1) * 512],
                                g[:, dc, ttt * 128:(ttt + 1) * 128],
                                wo_sb[:, dc, os_ * 512:(os_ + 1) * 512],
                                start=(dc == 0), stop=(dc == 7),
                            )
                    nc.vector.tensor_copy(out=y_sb[:, ttt, :], in_=y_ps[:])
                for ttt in range(2):
                    nc.sync.dma_start(y_d[u, ttt * 128:(ttt + 1) * 128, :], y_sb[:, ttt, :])

            for u in range(B):
                # ================= projections + RoPE =================
                qT = qkp.tile([128, S], f32r, tag="qT", name="qT")
                kT = qkp.tile([128, S], f32r, tag="kT", name="kT")
                v_sb = vp.tile([128, 16, 130], f32r, tag="v", name="v_sb")

                for tt in range(4):
                    ts = slice(tt * 512, (tt + 1) * 512)
                    xt_sb = xtp.tile([128, 8, 512], f32r, tag="xt", name="xt_sb")
                    for dc in range(8):
                        nc.sync.dma_start(
                            xt_sb[:, dc, :], xt_d[u, dc * 128:(dc + 1) * 128, ts]
                        )
                    if u == 0 and tt == 0:
                        # non-critical consts: emitted after the first xt tile
                        nc.gpsimd.dma_start(ident[:], ident_d[:])
                        nc.gpsimd.dma_start(ones_sb[:], ones_d[:])
                        nc.gpsimd.dma_start(cost[:], cost_d[:])
                        nc.gpsimd.dma_start(sinmt[:], sinmt_d[:])
                        nc.gpsimd.dma_start(masks[:, 0, :], masks_d[0])
                        nc.gpsimd.dma_start(masks[:, 1, :], masks_d[1])
                        nc.gpsimd.dma_start(identr[:], identr_d[:])
                    qk_ps = psp.tile([128, 1024], f32, tag="ps", name="qk_ps")
                    v_ps = psp.tile([128, 1024], f32, tag="ps", name="v_ps")
                    for dc in range(8):
                        st = dc == 0
                        sp = dc == 7
                        rhs = xt_sb[:, dc, :]
                        nc.tensor.matmul(qk_ps[:, 0:512], wq_sb[:, dc, :], rhs, start=st, stop=sp)
                        nc.tensor.matmul(qk_ps[:, 512:1024], wk_sb[:, dc, :], rhs, start=st, stop=sp)
                        nc.tensor.matmul(v_ps[:, 0:512], wv_sb[:, dc, :], rhs, start=st, stop=sp)

                    # RoPE: dst = q*cos + pairswap(q)*sinm
                    for src, dst in ((qk_ps[:, 0:512], qT), (qk_ps[:, 512:1024], kT)):
                        qs = rtmp.tile([128, 512], f32, tag="qs", name="qs")
                        t2 = rtmp.tile([128, 512], f32, tag="t2", name="t2")
                        nc.vector.stream_shuffle(qs[:], src, SWAP_MASK)
                        nc.vector.tensor_tensor(out=dst[:, ts], in0=src, in1=cost[:, ts], op=MUL)
                        nc.vector.tensor_tensor(out=t2[:], in0=qs[:], in1=sinmt[:, ts], op=MUL)
                        nc.vector.tensor_tensor(out=dst[:, ts], in0=dst[:, ts], in1=t2[:], op=ADD)

                    # v -> token-major via PE transpose; ones columns appended
                    vtmp = rtmp.tile([128, 512], f32, tag="vtmp", name="vtmp")
                    nc.vector.tensor_copy(out=vtmp[:], in_=v_ps[:, 0:512])
                    for s4 in range(4):
                        kt = tt * 4 + s4
                        tr = v_ps[:, 512 + s4 * 128: 512 + (s4 + 1) * 128]
                        nc.tensor.transpose(tr, vtmp[:, s4 * 128:(s4 + 1) * 128], ident[:])
                        dst = v_sb[:, kt, :].rearrange("p (u c) -> p u c", u=2)[:, :, 0:64]
                        src = tr.rearrange("p (u c) -> p u c", u=2)
                        nc.vector.tensor_copy(out=dst, in_=src)
                    nc.vector.tensor_copy(out=v_sb[:, tt * 4:(tt + 1) * 4, 64:65],
                                          in_=ones_sb[:, tt * 4:(tt + 1) * 4])
                    nc.vector.tensor_copy(out=v_sb[:, tt * 4:(tt + 1) * 4, 129:130],
                                          in_=ones_sb[:, tt * 4:(tt + 1) * 4])

                # ================= attention (normalize+ship per q-tile) =================
                aoT = aop.tile([128, S], f32r, tag="aoT", name="aoT")
                recip = smallp.tile([1, 8, 512], f32, tag="recip", name="recip")
                for qi in range(4):
                    qsl = slice(qi * 512, (qi + 1) * 512)
                    outT = psp.tile([128, 1024], f32, tag="ps", name="outT")
                    n_kt = 4 * qi + 4
                    for kt in range(n_kt):
                        ksl = slice(kt * 128, (kt + 1) * 128)
                        diag_pos = kt - 4 * qi
                        sc = psp.tile([128, 1024], f32, tag="ps", name="sc")
                        at = attnp.tile([128, 1024], f32r, tag="at", name="at")
                        if diag_pos < 2:
                            for h in (0, 1):
                                hp = slice(h * 64, (h + 1) * 64)
                                nc.tensor.matmul(
                                    sc[:, h * 512:(h + 1) * 512],
                                    kT[hp, ksl],
                                    qT[hp, qsl],
                                    start=True, stop=(diag_pos < 0),
                                    skip_group_check=True,
                                )
                            if diag_pos >= 0:
                                # causal mask: accumulate -1e9 pattern via identity matmul
                                for h in (0, 1):
                                    nc.tensor.matmul(
                                        sc[:, h * 512: h * 512 + 256],
                                        identr[:],
                                        masks[:, diag_pos, :],
                                        start=False, stop=True,
                                        skip_group_check=True,
                                    )
                            nc.scalar.activation(out=at[:], in_=sc[:], func=EXP, scale=0.125)
                            for h in (0, 1):
                                nc.tensor.matmul(
                                    outT[0:65, h * 512:(h + 1) * 512],
                                    v_sb[:, kt, :].rearrange("p (u c) -> p u c", u=2)[:, h, :],
                                    at[:, h * 512:(h + 1) * 512],
                                    start=(kt == 0), stop=(kt == n_kt - 1),
                                    skip_group_check=True,
                                )
                        else:
                            # kt2/kt3 of the diagonal: only q columns 256:512
                            for h in (0, 1):
                                hp = slice(h * 64, (h + 1) * 64)
                                nc.tensor.matmul(
                                    sc[:, h * 512 + 256: h * 512 + 512],
                                    kT[hp, ksl],
                                    qT[hp, qsl][:, 256:512],
                                    start=True, stop=False,
                                    skip_group_check=True,
                                )
                                nc.tensor.matmul(
                                    sc[:, h * 512 + 256: h * 512 + 512],
                                    identr[:],
                                    masks[:, diag_pos - 2, :],
                                    start=False, stop=True,
                                    skip_group_check=True,
                                )
                            scs = sc.rearrange("p (h q) -> p h q", h=2)[:, :, 256:512]
                            ats = at.rearrange("p (h q) -> p h q", h=2)[:, :, 256:512]
                            nc.scalar.activation(out=ats, in_=scs, func=EXP, scale=0.125)
                            for h in (0, 1):
                                nc.tensor.matmul(
                                    outT[0:65, h * 512 + 256: h * 512 + 512],
                                    v_sb[:, kt, :].rearrange("p (u c) -> p u c", u=2)[:, h, :],
                                    at[:, h * 512 + 256: h * 512 + 512],
                                    start=False, stop=(kt == n_kt - 1),
                                    skip_group_check=True,
                                )
                    # unload outT; normalize + ship this q-tile immediately
                    dent = smallp.tile([1, 512], f32, tag="dent", name="dent", bufs=2)
                    for h in (0, 1):
                        nc.vector.tensor_copy(
                            out=aoT[h * 64:(h + 1) * 64, qsl],
                            in_=outT[0:64, h * 512:(h + 1) * 512],
                        )
                        nc.vector.tensor_copy(
                            out=dent[0:1, :], in_=outT[64:65, h * 512:(h + 1) * 512]
                        )
                        nc.vector.reciprocal_approx_fast(
                            out=recip[0:1, h * 4 + qi, :], in_=dent[0:1, :]
                        )
                        nc.sync.dma_start(
                            recip_dram[u, h * 4 + qi: h * 4 + qi + 1, :],
                            recip[0:1, h * 4 + qi, :],
                        )
                    rb = rbp.tile([128, 512], f32, tag="rb", name="rb")
                    for h in (0, 1):
                        nc.gpsimd.dma_start(
                            rb[h * 64:(h + 1) * 64, :],
                            recip_dram[u, h * 4 + qi: h * 4 + qi + 1, :].to_broadcast([64, 512]),
                        )
                    nc.vector.tensor_tensor(out=aoT[:, qsl], in0=aoT[:, qsl], in1=rb[:], op=MUL)
                    for s in (2 * qi, 2 * qi + 1):
                        nc.sync.dma_start(a2a_in[u][s], aoT[:, s * 256:(s + 1) * 256])

                if u == 0:
                    # O-projection weights: off the startup critical path
                    for dc in range(8):
                        nc.gpsimd.dma_start(wo_sb[:, dc, :], wot_d[dc * 128:(dc + 1) * 128, :])

                if u > 0:
                    # unit u-1's O-projection must be emitted BEFORE unit u's
                    # collective: reads of a2a_out[u-1] otherwise wait on this
                    # collective too (collective completions share one
                    # cumulative semaphore).
                    o_projection(u - 1)
                nc.gpsimd.collective_compute(
                    "AllToAll",
                    mybir.AluOpType.bypass,
                    replica_groups=[list(range(NC))],
                    ins=[a2a_in[u].opt()],
                    outs=[a2a_out[u].opt()],
                )

            o_projection(B - 1)

    nc.compile()
    return nc


def _host_inputs(x, wq, wk, wv, wo):
    xt = np.ascontiguousarray(x.transpose(0, 2, 1))
    wot = np.ascontiguousarray(wo.T)

    p = np.arange(128)
    invf = THETA ** (-2.0 * ((p % 64) // 2) / 64.0)
    ang = invf[:, None] * np.arange(S)[None, :]
    cost = np.cos(ang).astype(np.float32)
    sinmt = (np.sin(ang) * np.where(p % 2 == 0, -1.0, 1.0)[:, None]).astype(np.float32)

    i = np.arange(128)[:, None]
    j = np.arange(256)[None, :]
    # additive causal masks: 0 where allowed (j >= i + off), -1e9 where masked
    masks = np.stack([
        np.where(j >= i + 0, 0.0, -1e9).astype(np.float32),
        np.where(j >= i + 128, 0.0, -1e9).astype(np.float32),
    ])
    ident = np.eye(128, dtype=np.float32)

    in_maps = []
    for c in range(NC):
        sl = slice(c * 128, (c + 1) * 128)
        in_maps.append({
            "xt": xt,
            "wqt": np.ascontiguousarray(wq[sl, :].T),
            "wkt": np.ascontiguousarray(wk[sl, :].T),
            "wvt": np.ascontiguousarray(wv[sl, :].T),
            "wot": wot,
            "cost": cost,
            "sinmt": sinmt,
            "masks": masks,
            "ident": ident,
            "identr": ident,
            "ones": np.ones((128, 16), np.float32),
        })
    return in_maps


def kernel(x, wq, wk, wv, wo, _trace=False):
    from concourse.bass_utils import run_bass_kernel_spmd

    if "nc" not in _COMPILED:
        _COMPILED["nc"] = _build()
    nc = _COMPILED["nc"]

    in_maps = _host_inputs(
        np.asarray(x, np.float32), np.asarray(wq, np.float32),
        np.asarray(wk, np.float32), np.asarray(wv, np.float32),
        np.asarray(wo, np.float32),
    )
    res = run_bass_kernel_spmd(nc, in_maps, core_ids=list(range(NC)), trace=_trace)
    _COMPILED["last_result"] = res

    y = np.zeros((B, S, D), np.float32)
    for c in range(NC):
        yc = res.results[c]["y"]
        y[:, c * 256:(c + 1) * 256, :] = yc
    return y



# revision 4
# speedup vs baseline: 1.2746x; 1.2746x over previous
"""Multi-head self-attention (RoPE, causal) Bass kernel for 8 TRN2 NeuronCores.

Sharding: tensor-parallel over heads for QKV+attention (2 heads/core),
chunked AllToAll, then token-parallel O-projection (512 tokens/core).

bf16 data path (fp32 PSUM accumulation + fp32 softmax statistics):
  xt/wq/wk/wv/wo/qT/kT/v/at/aoT/a2a payload are bf16 -> FWL weight loads,
  half DMA + collective bytes. Measured rel err ~6e-3 (gate 2e-2).

Layouts (per core):
  qT/kT:    [128 part = 2 heads x 64 dk, t] bf16 (RoPE'd projections)
  scoresT:  [128 part = k-tile, q free] PSUM f32 (softmax sum via ones-row)
  v_sb:     [128 part = k-tile tokens, 130] bf16 ([v_h0 | ones | v_h1 | ones])
  aoT:      [128 d, 512] bf16 per q-tile, normalized on PSUM unload
  y:        [t, o] f32 token-major final output

Causal masking via -1e9 identity-matmul accumulation (PE, bf16).
Per-unit AllToAll is split in two [NC,128,128] chunks (tokens 0:1024 /
1024:2048) so the last collective + O-projection tail is short; chunk
O-projections are interleaved into the next unit's attention.
"""

import numpy as np

B, S, D, H, DK = 2, 2048, 1024, 16, 64
NC = 8
THETA = 10000.0

_COMPILED = {}


def _build():
    import concourse.bass as bass
    import concourse.tile as tile
    from concourse import bacc, mybir

    f32 = mybir.dt.float32
    bf16 = mybir.dt.bfloat16
    MUL = mybir.AluOpType.mult
    ADD = mybir.AluOpType.add
    EXP = mybir.ActivationFunctionType.Exp

    nc = bacc.Bacc(num_devices=NC)

    xt_d = nc.dram_tensor("xt", [B, D, S], bf16, kind="ExternalInput")
    wqt_d = nc.dram_tensor("wqt", [D, 128], bf16, kind="ExternalInput")
    wkt_d = nc.dram_tensor("wkt", [D, 128], bf16, kind="ExternalInput")
    wvt_d = nc.dram_tensor("wvt", [D, 128], bf16, kind="ExternalInput")
    wot_d = nc.dram_tensor("wot", [D, D], bf16, kind="ExternalInput")
    cost_d = nc.dram_tensor("cost", [128, S], f32, kind="ExternalInput")
    sinmt_d = nc.dram_tensor("sinmt", [128, S], f32, kind="ExternalInput")
    masks_d = nc.dram_tensor("masks", [2, 128, 256], bf16, kind="ExternalInput")
    ident_d = nc.dram_tensor("ident", [128, 128], f32, kind="ExternalInput")
    identb_d = nc.dram_tensor("identb", [128, 128], bf16, kind="ExternalInput")
    ones_d = nc.dram_tensor("ones", [128, 16], bf16, kind="ExternalInput")
    y_d = nc.dram_tensor("y", [B, 2, 128, D], f32, kind="ExternalOutput")

    SWAP_MASK = [(i ^ 1) for i in range(32)]

    with tile.TileContext(nc) as tc:
        with (
            tc.tile_pool(name="const", bufs=1) as constp,
            tc.tile_pool(name="xtp", bufs=2) as xtp,
            tc.tile_pool(name="qk", bufs=2) as qkp,
            tc.tile_pool(name="vp", bufs=2) as vp,
            tc.tile_pool(name="attn", bufs=3) as attnp,
            tc.tile_pool(name="ao", bufs=2) as aop,
            tc.tile_pool(name="small", bufs=1) as smallp,
            tc.tile_pool(name="rbp", bufs=2) as rbp,
            tc.tile_pool(name="rtmp", bufs=2) as rtmp,
            tc.tile_pool(name="oproj", bufs=2) as op_,
            tc.tile_pool(name="yp", bufs=2) as yp,
            tc.tile_pool(name="ps", bufs=4, space="PSUM") as psp,
            tc.tile_pool(name="dram", bufs=1, space="DRAM") as dramp,
        ):
            # ---- constant tiles ----
            cost = constp.tile([128, S], f32)
            sinmt = constp.tile([128, S], f32)
            masks = constp.tile([128, 2, 256], bf16)
            ident = constp.tile([128, 128], f32)
            identb = constp.tile([128, 128], bf16)
            ones_sb = constp.tile([128, 16], bf16)
            wq_sb = constp.tile([128, 8, 128], bf16)
            wk_sb = constp.tile([128, 8, 128], bf16)
            wv_sb = constp.tile([128, 8, 128], bf16)
            wo_sb = constp.tile([128, 8, D], bf16)

            # critical path: projection weights, one batched DMA each
            nc.sync.dma_start(wq_sb[:], wqt_d[:, :].rearrange("(dc p) c -> p dc c", dc=8))
            nc.sync.dma_start(wk_sb[:], wkt_d[:, :].rearrange("(dc p) c -> p dc c", dc=8))
            nc.sync.dma_start(wv_sb[:], wvt_d[:, :].rearrange("(dc p) c -> p dc c", dc=8))

            warm_in = dramp.tile([NC, 64], bf16, name="warm_in")
            warm_out = dramp.tile([NC, 64], bf16, name="warm_out")
            nc.gpsimd.collective_compute(
                "AllToAll",
                mybir.AluOpType.bypass,
                replica_groups=[list(range(NC))],
                ins=[warm_in.opt()],
                outs=[warm_out.opt()],
            )
            # chunked a2a: chunk 0 = tokens [0,1024), chunk 1 = [1024,2048)
            a2a_in = [
                [dramp.tile([NC, 128, 128], bf16, name=f"a2ai{u}_{c}") for c in range(2)]
                for u in range(B)
            ]
            a2a_out = [
                [dramp.tile([NC, 128, 128], bf16, name=f"a2ao{u}_{c}") for c in range(2)]
                for u in range(B)
            ]
            recip_dram = dramp.tile([B, 8, 512], f32)

            def collective(u, c):
                nc.gpsimd.collective_compute(
                    "AllToAll",
                    mybir.AluOpType.bypass,
                    replica_groups=[list(range(NC))],
                    ins=[a2a_in[u][c].opt()],
                    outs=[a2a_out[u][c].opt()],
                )

            def o_projection(u, c):
                g = op_.tile([128, 8, 128], bf16, tag="g", name="g")
                nc.sync.dma_start(g[:], a2a_out[u][c].rearrange("s p c -> p s c"))
                y_ps = psp.tile([128, 1024], f32, tag="ps", name="y_ps")
                for os_ in range(2):
                    for dc in range(8):
                        nc.tensor.matmul(
                            y_ps[:, os_ * 512:(os_ + 1) * 512],
                            g[:, dc, :],
                            wo_sb[:, dc, os_ * 512:(os_ + 1) * 512],
                            start=(dc == 0), stop=(dc == 7),
                            skip_group_check=True,
                        )
                y_sb = yp.tile([128, D], f32, tag="y", name="y_sb")
                nc.vector.tensor_copy(out=y_sb[:], in_=y_ps[:])
                nc.sync.dma_start(y_d[u, c], y_sb[:])

            for u in range(B):
                # ================= projections + RoPE =================
                qT = qkp.tile([128, S], bf16, tag="qT", name="qT")
                kT = qkp.tile([128, S], bf16, tag="kT", name="kT")
                v_sb = vp.tile([128, 16, 130], bf16, tag="v", name="v_sb")

                for tt in range(4):
                    ts = slice(tt * 512, (tt + 1) * 512)
                    xt_sb = xtp.tile([128, 8, 512], bf16, tag="xt", name="xt_sb")
                    src = xt_d[u, :, ts].rearrange("(dc p) s -> p dc s", dc=8)
                    nc.sync.dma_start(xt_sb[:, 0:4, :], src[:, 0:4, :])
                    nc.sync.dma_start(xt_sb[:, 4:8, :], src[:, 4:8, :])
                    if u == 0 and tt == 0:
                        # non-critical consts after the first xt tile
                        nc.gpsimd.dma_start(ident[:], ident_d[:])
                        nc.gpsimd.dma_start(identb[:], identb_d[:])
                        nc.gpsimd.dma_start(ones_sb[:], ones_d[:])
                        nc.gpsimd.dma_start(masks[:, 0, :], masks_d[0])
                        nc.gpsimd.dma_start(masks[:, 1, :], masks_d[1])
                        nc.gpsimd.dma_start(cost[:, 0:1024], cost_d[:, 0:1024])
                        nc.gpsimd.dma_start(sinmt[:, 0:1024], sinmt_d[:, 0:1024])
                        nc.gpsimd.dma_start(cost[:, 1024:2048], cost_d[:, 1024:2048])
                        nc.gpsimd.dma_start(sinmt[:, 1024:2048], sinmt_d[:, 1024:2048])
                    qk_ps = psp.tile([128, 1024], f32, tag="ps", name="qk_ps")
                    v_ps = psp.tile([128, 1024], f32, tag="ps", name="v_ps")
                    for dc in range(8):
                        st = dc == 0
                        sp = dc == 7
                        rhs = xt_sb[:, dc, :]
                        nc.tensor.matmul(qk_ps[:, 0:512], wq_sb[:, dc, :], rhs, start=st, stop=sp)
                        nc.tensor.matmul(qk_ps[:, 512:1024], wk_sb[:, dc, :], rhs, start=st, stop=sp)
                        nc.tensor.matmul(v_ps[:, 0:512], wv_sb[:, dc, :], rhs, start=st, stop=sp)

                    # RoPE: dst = q*cos + pairswap(q)*sinm  (bf16 out)
                    for src_, dst in ((qk_ps[:, 0:512], qT), (qk_ps[:, 512:1024], kT)):
                        qs = rtmp.tile([128, 512], f32, tag="qs", name="qs")
                        t2 = rtmp.tile([128, 512], bf16, tag="t2", name="t2")
                        nc.vector.stream_shuffle(qs[:], src_, SWAP_MASK)
                        nc.vector.tensor_tensor(out=dst[:, ts], in0=src_, in1=cost[:, ts], op=MUL)
                        nc.vector.tensor_tensor(out=t2[:], in0=qs[:], in1=sinmt[:, ts], op=MUL)
                        nc.vector.tensor_tensor(out=dst[:, ts], in0=dst[:, ts], in1=t2[:], op=ADD)

                    # v -> token-major via PE transpose; ones columns appended
                    vtmp = rtmp.tile([128, 512], f32, tag="vtmp", name="vtmp")
                    nc.vector.tensor_copy(out=vtmp[:], in_=v_ps[:, 0:512])
                    for s4 in range(4):
                        kt = tt * 4 + s4
                        tr = v_ps[:, 512 + s4 * 128: 512 + (s4 + 1) * 128]
                        nc.tensor.transpose(tr, vtmp[:, s4 * 128:(s4 + 1) * 128], ident[:])
                        dst = v_sb[:, kt, :].rearrange("p (u c) -> p u c", u=2)[:, :, 0:64]
                        src_ = tr.rearrange("p (u c) -> p u c", u=2)
                        nc.vector.tensor_copy(out=dst, in_=src_)
                    nc.vector.tensor_copy(out=v_sb[:, tt * 4:(tt + 1) * 4, 64:65],
                                          in_=ones_sb[:, tt * 4:(tt + 1) * 4])
                    nc.vector.tensor_copy(out=v_sb[:, tt * 4:(tt + 1) * 4, 129:130],
                                          in_=ones_sb[:, tt * 4:(tt + 1) * 4])

                # ================= attention =================
                recip = smallp.tile([1, 8, 512], f32, tag="recip", name="recip")
                for qi in range(4):
                    qsl = slice(qi * 512, (qi + 1) * 512)
                    outT = psp.tile([128, 1024], f32, tag="ps", name="outT")
                    n_kt = 4 * qi + 4
                    # software pipeline: AV for tile kt is emitted after
                    # scores/exp of tile kt+1 so the PE isn't stalled on ACT
                    pend = None  # (at, kt, full)

                    def flush_av(last):
                        nonlocal pend
                        if pend is None:
                            return
                        at_, kt_, full = pend
                        st = kt_ == 0
                        for h in (0, 1):
                            if full:
                                nc.tensor.matmul(
                                    outT[0:65, h * 512:(h + 1) * 512],
                                    v_sb[:, kt_, :].rearrange("p (u c) -> p u c", u=2)[:, h, :],
                                    at_[:, h * 512:(h + 1) * 512],
                                    start=st, stop=last,
                                    skip_group_check=True,
                                )
                            else:
                                nc.tensor.matmul(
                                    outT[0:65, h * 512 + 256: h * 512 + 512],
                                    v_sb[:, kt_, :].rearrange("p (u c) -> p u c", u=2)[:, h, :],
                                    at_[:, h * 512 + 256: h * 512 + 512],
                                    start=False, stop=last,
                                    skip_group_check=True,
                                )
                        pend = None

                    for kt in range(n_kt):
                        ksl = slice(kt * 128, (kt + 1) * 128)
                        diag_pos = kt - 4 * qi
                        sc = psp.tile([128, 1024], f32, tag="ps", name="sc")
                        at = attnp.tile([128, 1024], bf16, tag="at", name="at")
                        if diag_pos < 2:
                            for h in (0, 1):
                                hp = slice(h * 64, (h + 1) * 64)
                                nc.tensor.matmul(
                                    sc[:, h * 512:(h + 1) * 512],
                                    kT[hp, ksl],
                                    qT[hp, qsl],
                                    start=True, stop=(diag_pos < 0),
                                    skip_group_check=True,
                                )
                            if diag_pos >= 0:
                                # causal mask: -1e9 pattern via identity matmul
                                for h in (0, 1):
                                    nc.tensor.matmul(
                                        sc[:, h * 512: h * 512 + 256],
                                        identb[:],
                                        masks[:, diag_pos, :],
                                        start=False, stop=True,
                                        skip_group_check=True,
                                    )
                            flush_av(False)
                            nc.scalar.activation(out=at[:], in_=sc[:], func=EXP, scale=0.125)
                            pend = (at, kt, True)
                        else:
                            # kt2/kt3 of the diagonal: only q columns 256:512
                            for h in (0, 1):
                                hp = slice(h * 64, (h + 1) * 64)
                                nc.tensor.matmul(
                                    sc[:, h * 512 + 256: h * 512 + 512],
                                    kT[hp, ksl],
                                    qT[hp, qsl][:, 256:512],
                                    start=True, stop=False,
                                    skip_group_check=True,
                                )
                            for h in (0, 1):
                                nc.tensor.matmul(
                                    sc[:, h * 512 + 256: h * 512 + 512],
                                    identb[:],
                                    masks[:, diag_pos - 2, :],
                                    start=False, stop=True,
                                    skip_group_check=True,
                                )
                            flush_av(False)
                            scs = sc.rearrange("p (h q) -> p h q", h=2)[:, :, 256:512]
                            ats = at.rearrange("p (h q) -> p h q", h=2)[:, :, 256:512]
                            nc.scalar.activation(out=ats, in_=scs, func=EXP, scale=0.125)
                            pend = (at, kt, False)
                    flush_av(True)

                    # unload + normalize + ship this q-tile
                    aoT = aop.tile([128, 512], bf16, tag="aoT", name="aoT")
                    dent = smallp.tile([1, 512], f32, tag="dent", name="dent", bufs=2)
                    for h in (0, 1):
                        nc.vector.tensor_copy(
                            out=aoT[h * 64:(h + 1) * 64, :],
                            in_=outT[0:64, h * 512:(h + 1) * 512],
                        )
                        nc.vector.tensor_copy(
                            out=dent[0:1, :], in_=outT[64:65, h * 512:(h + 1) * 512]
                        )
                        nc.vector.reciprocal_approx_fast(
                            out=recip[0:1, h * 4 + qi, :], in_=dent[0:1, :]
                        )
                        nc.sync.dma_start(
                            recip_dram[u, h * 4 + qi: h * 4 + qi + 1, :],
                            recip[0:1, h * 4 + qi, :],
                        )
                    rb = rbp.tile([128, 512], f32, tag="rb", name="rb")
                    for h in (0, 1):
                        nc.gpsimd.dma_start(
                            rb[h * 64:(h + 1) * 64, :],
                            recip_dram[u, h * 4 + qi: h * 4 + qi + 1, :].to_broadcast([64, 512]),
                        )
                    nc.vector.tensor_tensor(out=aoT[:], in0=aoT[:], in1=rb[:], op=MUL)
                    # ship: chunk c = qi//2, slots 4*(qi%2) .. +4
                    c = qi // 2
                    j0 = 4 * (qi % 2)
                    nc.sync.dma_start(
                        a2a_in[u][c][j0:j0 + 4].rearrange("s p c -> p s c"),
                        aoT[:].rearrange("p (s c) -> p s c", s=4),
                    )
                    if u == 0 and qi == 1:
                        collective(0, 0)
                    elif u == 0 and qi == 3:
                        collective(0, 1)
                        # O-projection weights: off the startup critical path
                        for dc in range(8):
                            nc.gpsimd.dma_start(
                                wo_sb[:, dc, :], wot_d[dc * 128:(dc + 1) * 128, :]
                            )
                    elif u == 1 and qi == 0:
                        o_projection(0, 0)
                    elif u == 1 and qi == 1:
                        o_projection(0, 1)
                        collective(1, 0)
                    elif u == 1 and qi == 2:
                        o_projection(1, 0)
                    elif u == 1 and qi == 3:
                        collective(1, 1)

            o_projection(1, 1)

    nc.compile()
    return nc


def _host_inputs(x, wq, wk, wv, wo):
    import ml_dtypes

    bf = ml_dtypes.bfloat16
    xt = np.ascontiguousarray(x.transpose(0, 2, 1)).astype(bf)
    wot = np.ascontiguousarray(wo.T).astype(bf)

    p = np.arange(128)
    invf = THETA ** (-2.0 * ((p % 64) // 2) / 64.0)
    ang = invf[:, None] * np.arange(S)[None, :]
    cost = np.cos(ang).astype(np.float32)
    sinmt = (np.sin(ang) * np.where(p % 2 == 0, -1.0, 1.0)[:, None]).astype(np.float32)

    i = np.arange(128)[:, None]
    j = np.arange(256)[None, :]
    # additive causal masks: 0 where allowed (j >= i + off), -1e9 where masked
    masks = np.stack([
        np.where(j >= i + 0, 0.0, -1e9),
        np.where(j >= i + 128, 0.0, -1e9),
    ]).astype(bf)
    ident = np.eye(128, dtype=np.float32)

    in_maps = []
    for c in range(NC):
        sl = slice(c * 128, (c + 1) * 128)
        in_maps.append({
            "xt": xt,
            "wqt": np.ascontiguousarray(wq[sl, :].T).astype(bf),
            "wkt": np.ascontiguousarray(wk[sl, :].T).astype(bf),
            "wvt": np.ascontiguousarray(wv[sl, :].T).astype(bf),
            "wot": wot,
            "cost": cost,
            "sinmt": sinmt,
            "masks": masks,
            "ident": ident,
            "identb": ident.astype(bf),
            "ones": np.ones((128, 16), bf),
        })
    return in_maps


def kernel(x, wq, wk, wv, wo, _trace=False):
    from concourse.bass_utils import run_bass_kernel_spmd

    if "nc" not in _COMPILED:
        _COMPILED["nc"] = _build()
    nc = _COMPILED["nc"]

    in_maps = _host_inputs(
        np.asarray(x, np.float32), np.asarray(wq, np.float32),
        np.asarray(wk, np.float32), np.asarray(wv, np.float32),
        np.asarray(wo, np.float32),
    )
    res = run_bass_kernel_spmd(nc, in_maps, core_ids=list(range(NC)), trace=_trace)
    _COMPILED["last_result"] = res

    y = np.zeros((B, S, D), np.float32)
    for c in range(NC):
        yc = res.results[c]["y"]  # [B, 2, 128, D]
        for u in range(B):
            y[u, 128 * c: 128 * c + 128, :] = yc[u, 0]
            y[u, 1024 + 128 * c: 1024 + 128 * c + 128, :] = yc[u, 1]
    return y


# revision 15
# speedup vs baseline: 1.3216x; 1.0369x over previous
"""Multi-head self-attention (RoPE, causal) Bass kernel for 8 TRN2 NeuronCores.

Sharding: tensor-parallel over heads for QKV+attention (2 heads/core),
chunked AllToAll, then token-parallel O-projection (512 tokens/core).

bf16 data path (fp32 PSUM accumulation + fp32 softmax statistics):
  xt/wq/wk/wv/wo/qT/kT/v/at/aoT/a2a payload are bf16 -> FWL weight loads,
  half DMA + collective bytes. Measured rel err ~6e-3 (gate 2e-2).

Layouts (per core):
  qT/kT:    [128 part = 2 heads x 64 dk, t] bf16 (RoPE'd projections)
  scoresT:  [128 part = k-tile, q free] PSUM f32 (softmax sum via ones-row)
  v_sb:     [128 part = k-tile tokens, 130] bf16 ([v_h0 | ones | v_h1 | ones])
  aoT:      [128 d, 512] bf16 per q-tile, normalized on PSUM unload
  y:        [t, o] f32 token-major final output

Causal masking via -1e9 identity-matmul accumulation (PE, bf16).
Per-unit AllToAll is split in two [NC,128,128] chunks (tokens 0:1024 /
1024:2048) so the last collective + O-projection tail is short; chunk
O-projections are interleaved into the next unit's attention.
"""

import numpy as np

B, S, D, H, DK = 2, 2048, 1024, 16, 64
NC = 8
THETA = 10000.0

_COMPILED = {}


def _build():
    import concourse.bass as bass
    import concourse.tile as tile
    from concourse import bacc, mybir

    f32 = mybir.dt.float32
    f32r = mybir.dt.float32r
    bf16 = mybir.dt.bfloat16
    MUL = mybir.AluOpType.mult
    ADD = mybir.AluOpType.add
    EXP = mybir.ActivationFunctionType.Exp

    nc = bacc.Bacc(num_devices=NC)

    xt_d = nc.dram_tensor("xt", [B, D, S], bf16, kind="ExternalInput")
    wqt_d = nc.dram_tensor("wqt", [D, 128], bf16, kind="ExternalInput")
    wkt_d = nc.dram_tensor("wkt", [D, 128], bf16, kind="ExternalInput")
    wvt_d = nc.dram_tensor("wvt", [D, 128], bf16, kind="ExternalInput")
    wot_d = nc.dram_tensor("wot", [D, D], bf16, kind="ExternalInput")
    cost_d = nc.dram_tensor("cost", [128, S], bf16, kind="ExternalInput")
    sinmt_d = nc.dram_tensor("sinmt", [128, S], bf16, kind="ExternalInput")
    trimask_d = nc.dram_tensor("trimask", [128, 128], bf16, kind="ExternalInput")
    ident_d = nc.dram_tensor("ident", [128, 128], f32, kind="ExternalInput")
    identb_d = nc.dram_tensor("identb", [128, 128], bf16, kind="ExternalInput")
    ones_d = nc.dram_tensor("ones", [128, 16], f32r, kind="ExternalInput")
    y_d = nc.dram_tensor("y", [B, 2, 128, D], f32, kind="ExternalOutput")

    SWAP_MASK = [(i ^ 1) for i in range(32)]

    with tile.TileContext(nc) as tc:
        with (
            tc.tile_pool(name="const", bufs=1) as constp,
            tc.tile_pool(name="xtp", bufs=2) as xtp,
            tc.tile_pool(name="qk", bufs=2) as qkp,
            tc.tile_pool(name="vp", bufs=2) as vp,
            tc.tile_pool(name="attn", bufs=3) as attnp,
            tc.tile_pool(name="ao", bufs=2) as aop,
            tc.tile_pool(name="small", bufs=1) as smallp,
            tc.tile_pool(name="rbp", bufs=2) as rbp,
            tc.tile_pool(name="rtmp", bufs=2) as rtmp,
            tc.tile_pool(name="oproj", bufs=2) as op_,
            tc.tile_pool(name="yp", bufs=2) as yp,
            tc.tile_pool(name="ps", bufs=4, space="PSUM") as psp,
            tc.tile_pool(name="dram", bufs=1, space="DRAM") as dramp,
        ):
            # ---- constant tiles ----
            cost = constp.tile([128, S], bf16)
            sinmt = constp.tile([128, S], bf16)
            trimask = constp.tile([128, 128], bf16)
            ident = constp.tile([128, 128], f32)
            identb = constp.tile([128, 128], bf16)
            ones_sb = constp.tile([128, 16], f32r)
            wq_sb = constp.tile([128, 8, 128], bf16)
            wk_sb = constp.tile([128, 8, 128], bf16)
            wv_sb = constp.tile([128, 8, 128], bf16)
            wo_sb = constp.tile([128, 8, D], bf16)

            # critical path: projection weights, one batched DMA each
            nc.sync.dma_start(wq_sb[:], wqt_d[:, :].rearrange("(dc p) c -> p dc c", dc=8))
            nc.sync.dma_start(wk_sb[:], wkt_d[:, :].rearrange("(dc p) c -> p dc c", dc=8))
            nc.sync.dma_start(wv_sb[:], wvt_d[:, :].rearrange("(dc p) c -> p dc c", dc=8))

            warm_in = dramp.tile([NC, 64], bf16, name="warm_in")
            warm_out = dramp.tile([NC, 64], bf16, name="warm_out")
            nc.gpsimd.collective_compute(
                "AllToAll",
                mybir.AluOpType.bypass,
                replica_groups=[list(range(NC))],
                ins=[warm_in.opt()],
                outs=[warm_out.opt()],
            )
            # chunked a2a: chunk 0 = tokens [0,1024), chunk 1 = [1024,2048)
            a2a_in = [
                [dramp.tile([NC, 128, 128], bf16, name=f"a2ai{u}_{c}") for c in range(2)]
                for u in range(B)
            ]
            a2a_out = [
                [dramp.tile([NC, 128, 128], bf16, name=f"a2ao{u}_{c}") for c in range(2)]
                for u in range(B)
            ]
            recip_dram = dramp.tile([B, 8, 512], f32)

            def collective(u, c):
                nc.gpsimd.collective_compute(
                    "AllToAll",
                    mybir.AluOpType.bypass,
                    replica_groups=[list(range(NC))],
                    ins=[a2a_in[u][c].opt()],
                    outs=[a2a_out[u][c].opt()],
                )

            def o_projection(u, c):
                g = op_.tile([128, 8, 128], bf16, tag="g", name="g")
                nc.sync.dma_start(g[:], a2a_out[u][c].rearrange("s p c -> p s c"))
                y_ps = psp.tile([128, 1024], f32, tag="ps", name="y_ps")
                for os_ in range(2):
                    for dc in range(8):
                        nc.tensor.matmul(
                            y_ps[:, os_ * 512:(os_ + 1) * 512],
                            g[:, dc, :],
                            wo_sb[:, dc, os_ * 512:(os_ + 1) * 512],
                            start=(dc == 0), stop=(dc == 7),
                            skip_group_check=True,
                        )
                y_sb = yp.tile([128, D], f32, tag="y", name="y_sb")
                nc.vector.tensor_copy(out=y_sb[:], in_=y_ps[:])
                nc.sync.dma_start(y_d[u, c], y_sb[:])

            for u in range(B):
                # ================= projections + RoPE =================
                qT = qkp.tile([128, S], bf16, tag="qT", name="qT")
                kT = qkp.tile([128, S], bf16, tag="kT", name="kT")
                v_sb = vp.tile([128, 16, 130], f32r, tag="v", name="v_sb")

                for tt in range(4):
                    ts = slice(tt * 512, (tt + 1) * 512)
                    xt_sb = xtp.tile([128, 8, 512], bf16, tag="xt", name="xt_sb")
                    src = xt_d[u, :, ts].rearrange("(dc p) s -> p dc s", dc=8)
                    nc.sync.dma_start(xt_sb[:, 0:4, :], src[:, 0:4, :])
                    nc.sync.dma_start(xt_sb[:, 4:8, :], src[:, 4:8, :])
                    if u == 0 and tt == 0:
                        # non-critical consts after the first xt tile
                        nc.gpsimd.dma_start(cost[:, 0:1024], cost_d[:, 0:1024])
                        nc.gpsimd.dma_start(sinmt[:, 0:1024], sinmt_d[:, 0:1024])
                        nc.gpsimd.dma_start(ident[:], ident_d[:])
                        nc.gpsimd.dma_start(identb[:], identb_d[:])
                        nc.gpsimd.dma_start(ones_sb[:], ones_d[:])
                        nc.gpsimd.dma_start(trimask[:], trimask_d[:])
                        nc.gpsimd.dma_start(cost[:, 1024:2048], cost_d[:, 1024:2048])
                        nc.gpsimd.dma_start(sinmt[:, 1024:2048], sinmt_d[:, 1024:2048])
                    qk_ps = psp.tile([128, 1024], f32, tag="ps", name="qk_ps")
                    v_ps = psp.tile([128, 1024], f32, tag="ps", name="v_ps")
                    for dc in range(8):
                        st = dc == 0
                        sp = dc == 7
                        rhs = xt_sb[:, dc, :]
                        nc.tensor.matmul(qk_ps[:, 0:512], wq_sb[:, dc, :], rhs, start=st, stop=sp)
                        nc.tensor.matmul(qk_ps[:, 512:1024], wk_sb[:, dc, :], rhs, start=st, stop=sp)
                        nc.tensor.matmul(v_ps[:, 0:512], wv_sb[:, dc, :], rhs, start=st, stop=sp)

                    # RoPE: dst = q*cos + pairswap(q)*sinm  (bf16 out)
                    for src_, dst in ((qk_ps[:, 0:512], qT), (qk_ps[:, 512:1024], kT)):
                        qs = rtmp.tile([128, 512], f32, tag="qs", name="qs")
                        t2 = rtmp.tile([128, 512], bf16, tag="t2", name="t2")
                        nc.vector.stream_shuffle(qs[:], src_, SWAP_MASK)
                        nc.vector.tensor_tensor(out=dst[:, ts], in0=src_, in1=cost[:, ts], op=MUL)
                        nc.vector.tensor_tensor(out=t2[:], in0=qs[:], in1=sinmt[:, ts], op=MUL)
                        nc.vector.tensor_tensor(out=dst[:, ts], in0=dst[:, ts], in1=t2[:], op=ADD)

                    # v -> token-major via PE transpose; ones columns appended
                    vtmp = rtmp.tile([128, 512], f32, tag="vtmp", name="vtmp")
                    nc.vector.tensor_copy(out=vtmp[:], in_=v_ps[:, 0:512])
                    for s4 in range(4):
                        kt = tt * 4 + s4
                        tr = v_ps[:, 512 + s4 * 128: 512 + (s4 + 1) * 128]
                        nc.tensor.transpose(tr, vtmp[:, s4 * 128:(s4 + 1) * 128], ident[:])
                        dst = v_sb[:, kt, :].rearrange("p (u c) -> p u c", u=2)[:, :, 0:64]
                        src_ = tr.rearrange("p (u c) -> p u c", u=2)
                        nc.vector.tensor_copy(out=dst, in_=src_)
                    nc.vector.tensor_copy(out=v_sb[:, tt * 4:(tt + 1) * 4, 64:65],
                                          in_=ones_sb[:, tt * 4:(tt + 1) * 4])
                    nc.vector.tensor_copy(out=v_sb[:, tt * 4:(tt + 1) * 4, 129:130],
                                          in_=ones_sb[:, tt * 4:(tt + 1) * 4])

                # ================= attention =================
                recip = smallp.tile([1, 8, 512], f32, tag="recip", name="recip")
                for qi in range(4):
                    qsl = slice(qi * 512, (qi + 1) * 512)
                    outT = psp.tile([128, 1024], f32, tag="ps", name="outT")
                    n_kt = 4 * qi + 4
                    # software pipeline (depth 2): AV for tile kt is emitted
                    # after scores/exp of tile kt+2 so the PE isn't stalled
                    # on the ACT engine's exp
                    pend = []  # (at, kt, q0)

                    def flush_av(last):
                        at_, kt_, q0 = pend.pop(0)
                        for h in (0, 1):
                            nc.tensor.matmul(
                                outT[0:65, h * 512 + q0:(h + 1) * 512],
                                v_sb[:, kt_, :].rearrange("p (u c) -> p u c", u=2)[:, h, :],
                                at_[:, h * 512 + q0:(h + 1) * 512],
                                start=(kt_ == 0), stop=last,
                                skip_group_check=True,
                            )

                    for kt in range(n_kt):
                        ksl = slice(kt * 128, (kt + 1) * 128)
                        rel = kt - 4 * qi  # >=0 on the diagonal block
                        q0 = 128 * rel if rel > 0 else 0
                        sc = psp.tile([128, 1024], f32, tag="ps", name="sc")
                        at = attnp.tile([128, 1024], f32r, tag="at", name="at")
                        for h in (0, 1):
                            hp = slice(h * 64, (h + 1) * 64)
                            nc.tensor.matmul(
                                sc[:, h * 512 + q0:(h + 1) * 512],
                                kT[hp, ksl],
                                qT[hp, qsl][:, q0:512],
                                start=True, stop=(rel < 0),
                                skip_group_check=True,
                            )
                        if rel >= 0:
                            # causal triangle at q columns [q0, q0+128)
                            for h in (0, 1):
                                nc.tensor.matmul(
                                    sc[:, h * 512 + q0: h * 512 + q0 + 128],
                                    identb[:],
                                    trimask[:],
                                    start=False, stop=True,
                                    skip_group_check=True,
                                )
                        if len(pend) >= 2:
                            flush_av(False)
                        if q0 == 0:
                            nc.scalar.activation(out=at[:], in_=sc[:], func=EXP, scale=0.125)
                        else:
                            scs = sc.rearrange("p (h q) -> p h q", h=2)[:, :, q0:512]
                            ats = at.rearrange("p (h q) -> p h q", h=2)[:, :, q0:512]
                            nc.scalar.activation(out=ats, in_=scs, func=EXP, scale=0.125)
                        pend.append((at, kt, q0))
                    while len(pend) > 1:
                        flush_av(False)
                    flush_av(True)

                    # unload + normalize + ship this q-tile
                    aoT = aop.tile([128, 512], bf16, tag="aoT", name="aoT")
                    dent = smallp.tile([1, 512], f32, tag="dent", name="dent", bufs=2)
                    for h in (0, 1):
                        nc.vector.tensor_copy(
                            out=aoT[h * 64:(h + 1) * 64, :],
                            in_=outT[0:64, h * 512:(h + 1) * 512],
                        )
                        nc.vector.tensor_copy(
                            out=dent[0:1, :], in_=outT[64:65, h * 512:(h + 1) * 512]
                        )
                        nc.vector.reciprocal_approx_fast(
                            out=recip[0:1, h * 4 + qi, :], in_=dent[0:1, :]
                        )
                        nc.sync.dma_start(
                            recip_dram[u, h * 4 + qi: h * 4 + qi + 1, :],
                            recip[0:1, h * 4 + qi, :],
                        )
                    rb = rbp.tile([128, 512], f32, tag="rb", name="rb")
                    for h in (0, 1):
                        nc.gpsimd.dma_start(
                            rb[h * 64:(h + 1) * 64, :],
                            recip_dram[u, h * 4 + qi: h * 4 + qi + 1, :].to_broadcast([64, 512]),
                        )
                    nc.vector.tensor_tensor(out=aoT[:], in0=aoT[:], in1=rb[:], op=MUL)
                    # ship: chunk c = qi//2, slots 4*(qi%2) .. +4
                    c = qi // 2
                    j0 = 4 * (qi % 2)
                    nc.sync.dma_start(
                        a2a_in[u][c][j0:j0 + 4].rearrange("s p c -> p s c"),
                        aoT[:].rearrange("p (s c) -> p s c", s=4),
                    )
                    if u == 0 and qi == 1:
                        collective(0, 0)
                    elif u == 0 and qi == 3:
                        collective(0, 1)
                        # O-projection weights: off the startup critical path
                        for dc in range(8):
                            nc.gpsimd.dma_start(
                                wo_sb[:, dc, :], wot_d[dc * 128:(dc + 1) * 128, :]
                            )
                    elif u == 1 and qi == 0:
                        o_projection(0, 0)
                    elif u == 1 and qi == 1:
                        o_projection(0, 1)
                        collective(1, 0)
                    elif u == 1 and qi == 3:
                        collective(1, 1)

            # tail: (1,0)'s collective finished during qi2/qi3, so these 32
            # matmuls overlap the last collective's skew + transfer
            o_projection(1, 0)
            o_projection(1, 1)

    nc.compile()
    return nc


def _host_inputs(x, wq, wk, wv, wo):
    import ml_dtypes

    bf = ml_dtypes.bfloat16
    xt = np.ascontiguousarray(x.transpose(0, 2, 1)).astype(bf)
    wot = np.ascontiguousarray(wo.T).astype(bf)

    p = np.arange(128)
    invf = THETA ** (-2.0 * ((p % 64) // 2) / 64.0)
    ang = invf[:, None] * np.arange(S)[None, :]
    cost = np.cos(ang).astype(bf)
    sinmt = (np.sin(ang) * np.where(p % 2 == 0, -1.0, 1.0)[:, None]).astype(bf)

    i = np.arange(128)[:, None]
    j = np.arange(128)[None, :]
    # additive causal triangle: 0 where allowed (j >= i), -1e9 where masked
    trimask = np.where(j >= i, 0.0, -1e9).astype(bf)
    ident = np.eye(128, dtype=np.float32)

    in_maps = []
    for c in range(NC):
        sl = slice(c * 128, (c + 1) * 128)
        in_maps.append({
            "xt": xt,
            "wqt": np.ascontiguousarray(wq[sl, :].T).astype(bf),
            "wkt": np.ascontiguousarray(wk[sl, :].T).astype(bf),
            "wvt": np.ascontiguousarray(wv[sl, :].T).astype(bf),
            "wot": wot,
            "cost": cost,
            "sinmt": sinmt,
            "trimask": trimask,
            "ident": ident,
            "identb": ident.astype(bf),
            "ones": np.ones((128, 16), np.float32),
        })
    return in_maps


def kernel(x, wq, wk, wv, wo, _trace=False):
    from concourse.bass_utils import run_bass_kernel_spmd

    if "nc" not in _COMPILED:
        _COMPILED["nc"] = _build()
    nc = _COMPILED["nc"]

    in_maps = _host_inputs(
        np.asarray(x, np.float32), np.asarray(wq, np.float32),
        np.asarray(wk, np.float32), np.asarray(wv, np.float32),
        np.asarray(wo, np.float32),
    )
    res = run_bass_kernel_spmd(nc, in_maps, core_ids=list(range(NC)), trace=_trace)
    _COMPILED["last_result"] = res

    y = np.zeros((B, S, D), np.float32)
    for c in range(NC):
        yc = res.results[c]["y"]  # [B, 2, 128, D]
        for u in range(B):
            y[u, 128 * c: 128 * c + 128, :] = yc[u, 0]
            y[u, 1024 + 128 * c: 1024 + 128 * c + 128, :] = yc[u, 1]
    return y


# revision 33
# speedup vs baseline: 1.3589x; 1.0282x over previous
"""Multi-head self-attention (RoPE, causal) Bass kernel for 8 TRN2 NeuronCores.

Sharding: tensor-parallel over heads for QKV+attention (2 heads/core),
chunked AllToAll, then token-parallel O-projection (512 tokens/core).

bf16 data path (fp32 PSUM accumulation + fp32 softmax statistics):
  xt/wq/wk/wv/wo/qT/kT/v/at/aoT/a2a payload are bf16 -> FWL weight loads,
  half DMA + collective bytes. Measured rel err ~6e-3 (gate 2e-2).

Layouts (per core):
  qT/kT:    [128 part = 2 heads x 64 dk, t] bf16 (RoPE'd projections)
  scoresT:  [128 part = k-tile, q free] PSUM f32 (softmax sum via ones-row)
  v_sb:     [128 part = k-tile tokens, 130] bf16 ([v_h0 | ones | v_h1 | ones])
  aoT:      [128 d, 512] bf16 per q-tile, normalized on PSUM unload
  y:        [t, o] f32 token-major final output

Causal masking via -1e9 identity-matmul accumulation (PE, bf16).
Per-unit AllToAll is split in two [NC,128,128] chunks (tokens 0:1024 /
1024:2048) so the last collective + O-projection tail is short; chunk
O-projections are interleaved into the next unit's attention.
"""

import numpy as np

B, S, D, H, DK = 2, 2048, 1024, 16, 64
NC = 8
THETA = 10000.0

_COMPILED = {}


def _build():
    import concourse.bass as bass
    import concourse.tile as tile
    from concourse import bacc, mybir

    f32 = mybir.dt.float32
    f32r = mybir.dt.float32r
    bf16 = mybir.dt.bfloat16
    MUL = mybir.AluOpType.mult
    ADD = mybir.AluOpType.add
    EXP = mybir.ActivationFunctionType.Exp

    nc = bacc.Bacc(num_devices=NC)

    xt_d = nc.dram_tensor("xt", [B, D, S], bf16, kind="ExternalInput")
    wqt_d = nc.dram_tensor("wqt", [D, 128], bf16, kind="ExternalInput")
    wkt_d = nc.dram_tensor("wkt", [D, 128], bf16, kind="ExternalInput")
    wvt_d = nc.dram_tensor("wvt", [D, 128], bf16, kind="ExternalInput")
    wot_d = nc.dram_tensor("wot", [D, D], bf16, kind="ExternalInput")
    cost_d = nc.dram_tensor("cost", [128, S], bf16, kind="ExternalInput")
    sinmt_d = nc.dram_tensor("sinmt", [128, S], bf16, kind="ExternalInput")
    trimask_d = nc.dram_tensor("trimask", [128, 128], bf16, kind="ExternalInput")
    ident_d = nc.dram_tensor("ident", [128, 128], f32, kind="ExternalInput")
    identb_d = nc.dram_tensor("identb", [128, 128], bf16, kind="ExternalInput")
    ones_d = nc.dram_tensor("ones", [128, 16], f32r, kind="ExternalInput")
    sel_d = nc.dram_tensor("sel", [1, 2, 128], f32r, kind="ExternalInput")
    y_d = nc.dram_tensor("y", [B, 2, 128, D], f32, kind="ExternalOutput")

    SWAP_MASK = [(i ^ 1) for i in range(32)]

    with tile.TileContext(nc) as tc:
        with (
            tc.tile_pool(name="const", bufs=1) as constp,
            tc.tile_pool(name="xtp", bufs=4) as xtp,
            tc.tile_pool(name="qk", bufs=2) as qkp,
            tc.tile_pool(name="vp", bufs=2) as vp,
            tc.tile_pool(name="attn", bufs=3) as attnp,
            tc.tile_pool(name="ao", bufs=2) as aop,
            tc.tile_pool(name="small", bufs=1) as smallp,
            tc.tile_pool(name="rtmp", bufs=2) as rtmp,
            tc.tile_pool(name="oproj", bufs=2) as op_,
            tc.tile_pool(name="yp", bufs=2) as yp,
            tc.tile_pool(name="ps", bufs=4, space="PSUM") as psp,
            tc.tile_pool(name="dram", bufs=1, space="DRAM") as dramp,
        ):
            # ---- constant tiles ----
            cost = constp.tile([128, S], bf16)
            sinmt = constp.tile([128, S], bf16)
            trimask = constp.tile([128, 128], bf16)
            ident = constp.tile([128, 128], f32)
            identb = constp.tile([128, 128], bf16)
            ones_sb = constp.tile([128, 16], f32r)
            sel_sb = constp.tile([1, 2, 128], f32r)
            wq_sb = constp.tile([128, 8, 128], bf16)
            wk_sb = constp.tile([128, 8, 128], bf16)
            wv_sb = constp.tile([128, 8, 128], bf16)
            wo_sb = constp.tile([128, 8, D], bf16)

            # critical path: projection weights, one batched DMA each
            nc.sync.dma_start(wq_sb[:], wqt_d[:, :].rearrange("(dc p) c -> p dc c", dc=8))
            nc.sync.dma_start(wk_sb[:], wkt_d[:, :].rearrange("(dc p) c -> p dc c", dc=8))
            nc.sync.dma_start(wv_sb[:], wvt_d[:, :].rearrange("(dc p) c -> p dc c", dc=8))

            warm_in = dramp.tile([NC, 64], bf16, name="warm_in")
            warm_out = dramp.tile([NC, 64], bf16, name="warm_out")
            nc.gpsimd.collective_compute(
                "AllToAll",
                mybir.AluOpType.bypass,
                replica_groups=[list(range(NC))],
                ins=[warm_in.opt()],
                outs=[warm_out.opt()],
            )
            # chunked a2a: chunk 0 = tokens [0,1024), chunk 1 = [1024,2048)
            a2a_in = [
                [dramp.tile([NC, 128, 128], bf16, name=f"a2ai{u}_{c}") for c in range(2)]
                for u in range(B)
            ]
            a2a_out = [
                [dramp.tile([NC, 128, 128], bf16, name=f"a2ao{u}_{c}") for c in range(2)]
                for u in range(B)
            ]
            def collective(u, c):
                nc.gpsimd.collective_compute(
                    "AllToAll",
                    mybir.AluOpType.bypass,
                    replica_groups=[list(range(NC))],
                    ins=[a2a_in[u][c].opt()],
                    outs=[a2a_out[u][c].opt()],
                )

            def o_projection(u, c):
                g = op_.tile([128, 8, 128], bf16, tag="g", name="g")
                nc.sync.dma_start(g[:], a2a_out[u][c].rearrange("s p c -> p s c"))
                y_ps = psp.tile([128, 1024], f32, tag="ps", name="y_ps")
                for os_ in range(2):
                    for dc in range(8):
                        nc.tensor.matmul(
                            y_ps[:, os_ * 512:(os_ + 1) * 512],
                            g[:, dc, :],
                            wo_sb[:, dc, os_ * 512:(os_ + 1) * 512],
                            start=(dc == 0), stop=(dc == 7),
                            skip_group_check=True,
                        )
                y_sb = yp.tile([128, D], f32, tag="y", name="y_sb")
                nc.vector.tensor_copy(out=y_sb[:], in_=y_ps[:])
                nc.sync.dma_start(y_d[u, c], y_sb[:])

            def load_xt(u, first):
                tiles = []
                for tt in range(4):
                    ts = slice(tt * 512, (tt + 1) * 512)
                    xt_sb = xtp.tile([128, 8, 512], bf16, tag="xt", name="xt_sb")
                    src = xt_d[u, :, ts].rearrange("(dc p) s -> p dc s", dc=8)
                    nc.sync.dma_start(xt_sb[:, 0:4, :], src[:, 0:4, :])
                    nc.sync.dma_start(xt_sb[:, 4:8, :], src[:, 4:8, :])
                    if first and tt == 0:
                        # non-critical consts after the first xt tile
                        nc.gpsimd.dma_start(cost[:, 0:1024], cost_d[:, 0:1024])
                        nc.gpsimd.dma_start(sinmt[:, 0:1024], sinmt_d[:, 0:1024])
                        nc.gpsimd.dma_start(ident[:], ident_d[:])
                        nc.gpsimd.dma_start(identb[:], identb_d[:])
                        nc.gpsimd.dma_start(ones_sb[:], ones_d[:])
                        nc.gpsimd.dma_start(sel_sb[:], sel_d[:])
                        nc.gpsimd.dma_start(trimask[:], trimask_d[:])
                        nc.gpsimd.dma_start(cost[:, 1024:2048], cost_d[:, 1024:2048])
                        nc.gpsimd.dma_start(sinmt[:, 1024:2048], sinmt_d[:, 1024:2048])
                    tiles.append(xt_sb)
                return tiles

            xts = {0: load_xt(0, True)}

            for u in range(B):
                # ================= projections + RoPE =================
                qT = qkp.tile([128, S], bf16, tag="qT", name="qT")
                kT = qkp.tile([128, S], bf16, tag="kT", name="kT")
                v_sb = vp.tile([128, 16, 130], f32r, tag="v", name="v_sb")

                for tt in range(4):
                    ts = slice(tt * 512, (tt + 1) * 512)
                    xt_sb = xts[u][tt]
                    qk_ps = psp.tile([128, 1024], f32, tag="ps", name="qk_ps")
                    v_ps = psp.tile([128, 1024], f32, tag="ps", name="v_ps")
                    for dc in range(8):
                        st = dc == 0
                        sp = dc == 7
                        rhs = xt_sb[:, dc, :]
                        nc.tensor.matmul(qk_ps[:, 0:512], wq_sb[:, dc, :], rhs, start=st, stop=sp)
                        nc.tensor.matmul(qk_ps[:, 512:1024], wk_sb[:, dc, :], rhs, start=st, stop=sp)
                        nc.tensor.matmul(v_ps[:, 0:512], wv_sb[:, dc, :], rhs, start=st, stop=sp)

                    # RoPE: dst = q*cos + pairswap(q)*sinm  (bf16 out)
                    for src_, dst in ((qk_ps[:, 0:512], qT), (qk_ps[:, 512:1024], kT)):
                        qs = rtmp.tile([128, 512], f32, tag="qs", name="qs")
                        t2 = rtmp.tile([128, 512], bf16, tag="t2", name="t2")
                        nc.vector.stream_shuffle(qs[:], src_, SWAP_MASK)
                        nc.vector.tensor_tensor(out=dst[:, ts], in0=src_, in1=cost[:, ts], op=MUL)
                        nc.vector.tensor_tensor(out=t2[:], in0=qs[:], in1=sinmt[:, ts], op=MUL)
                        nc.vector.tensor_tensor(out=dst[:, ts], in0=dst[:, ts], in1=t2[:], op=ADD)

                    # v -> token-major via PE transpose; ones columns appended
                    vtmp = rtmp.tile([128, 512], f32, tag="vtmp", name="vtmp")
                    nc.vector.tensor_copy(out=vtmp[:], in_=v_ps[:, 0:512])
                    for s4 in range(4):
                        kt = tt * 4 + s4
                        tr = v_ps[:, 512 + s4 * 128: 512 + (s4 + 1) * 128]
                        nc.tensor.transpose(tr, vtmp[:, s4 * 128:(s4 + 1) * 128], ident[:])
                        dst = v_sb[:, kt, :].rearrange("p (u c) -> p u c", u=2)[:, :, 0:64]
                        src_ = tr.rearrange("p (u c) -> p u c", u=2)
                        nc.vector.tensor_copy(out=dst, in_=src_)
                    nc.vector.tensor_copy(out=v_sb[:, tt * 4:(tt + 1) * 4, 64:65],
                                          in_=ones_sb[:, tt * 4:(tt + 1) * 4])
                    nc.vector.tensor_copy(out=v_sb[:, tt * 4:(tt + 1) * 4, 129:130],
                                          in_=ones_sb[:, tt * 4:(tt + 1) * 4])

                if u == 0:
                    # prefetch unit 1's activations during unit 0's attention
                    xts[1] = load_xt(1, False)

                # ================= attention =================
                for qi in range(4):
                    qsl = slice(qi * 512, (qi + 1) * 512)
                    outT = psp.tile([128, 1024], f32, tag="ps", name="outT")
                    n_kt = 4 * qi + 4
                    # software pipeline (depth 2): AV for tile kt is emitted
                    # after scores/exp of tile kt+2 so the PE isn't stalled
                    # on the ACT engine's exp
                    pend = []  # (at, kt, q0)

                    def flush_av(last):
                        at_, kt_, q0 = pend.pop(0)
                        for h in (0, 1):
                            nc.tensor.matmul(
                                outT[0:65, h * 512 + q0:(h + 1) * 512],
                                v_sb[:, kt_, :].rearrange("p (u c) -> p u c", u=2)[:, h, :],
                                at_[:, h * 512 + q0:(h + 1) * 512],
                                start=(kt_ == 0), stop=last,
                                skip_group_check=True,
                            )

                    for kt in range(n_kt):
                        ksl = slice(kt * 128, (kt + 1) * 128)
                        rel = kt - 4 * qi  # >=0 on the diagonal block
                        q0 = 128 * rel if rel > 0 else 0
                        sc = psp.tile([128, 1024], f32, tag="ps", name="sc")
                        at = attnp.tile([128, 1024], f32r, tag="at", name="at")
                        for h in (0, 1):
                            hp = slice(h * 64, (h + 1) * 64)
                            nc.tensor.matmul(
                                sc[:, h * 512 + q0:(h + 1) * 512],
                                kT[hp, ksl],
                                qT[hp, qsl][:, q0:512],
                                start=True, stop=(rel < 0),
                                skip_group_check=True,
                            )
                        if rel >= 0:
                            # causal triangle at q columns [q0, q0+128)
                            for h in (0, 1):
                                nc.tensor.matmul(
                                    sc[:, h * 512 + q0: h * 512 + q0 + 128],
                                    identb[:],
                                    trimask[:],
                                    start=False, stop=True,
                                    skip_group_check=True,
                                )
                        if len(pend) >= 2:
                            flush_av(False)
                        if q0 == 0:
                            nc.scalar.activation(out=at[:], in_=sc[:], func=EXP, scale=0.125)
                        else:
                            scs = sc.rearrange("p (h q) -> p h q", h=2)[:, :, q0:512]
                            ats = at.rearrange("p (h q) -> p h q", h=2)[:, :, q0:512]
                            nc.scalar.activation(out=ats, in_=scs, func=EXP, scale=0.125)
                        pend.append((at, kt, q0))
                    while len(pend) > 1:
                        flush_av(False)
                    flush_av(True)

                    # unload + normalize + ship this q-tile.  1/den per head is
                    # broadcast across 64 partitions by a K=2 selector matmul
                    # into the (now free) last score tile's PSUM region.
                    aoT = aop.tile([128, 512], bf16, tag="aoT", name="aoT")
                    rb = sc[:, 0:512]
                    for h in (0, 1):
                        nc.vector.tensor_copy(
                            out=aoT[h * 64:(h + 1) * 64, :],
                            in_=outT[0:64, h * 512:(h + 1) * 512],
                        )
                        dent = smallp.tile([1, 512], f32, tag=f"dent{h}",
                                           name="dent", bufs=2)
                        rc = smallp.tile([1, 512], f32, tag=f"rc{h}",
                                         name="rc", bufs=2)
                        nc.vector.tensor_copy(
                            out=dent[0:1, :], in_=outT[64:65, h * 512:(h + 1) * 512]
                        )
                        nc.vector.reciprocal_approx_fast(out=rc[0:1, :], in_=dent[0:1, :])
                        rcr = smallp.tile([1, 512], f32r, tag=f"rcr{h}",
                                          name="rcr", bufs=2)
                        nc.vector.tensor_copy(out=rcr[0:1, :], in_=rc[0:1, :])
                        nc.tensor.matmul(
                            rb, sel_sb[:, h, :], rcr[0:1, :],
                            start=(h == 0), stop=(h == 1), skip_group_check=True,
                        )
                    nc.vector.tensor_tensor(out=aoT[:], in0=aoT[:], in1=rb, op=MUL)
                    # ship: chunk c = qi//2, slots 4*(qi%2) .. +4
                    c = qi // 2
                    j0 = 4 * (qi % 2)
                    nc.sync.dma_start(
                        a2a_in[u][c][j0:j0 + 4].rearrange("s p c -> p s c"),
                        aoT[:].rearrange("p (s c) -> p s c", s=4),
                    )
                    if u == 0 and qi == 1:
                        collective(0, 0)
                    elif u == 0 and qi == 3:
                        collective(0, 1)
                        # O-projection weights: off the startup critical path
                        for dc in range(8):
                            nc.gpsimd.dma_start(
                                wo_sb[:, dc, :], wot_d[dc * 128:(dc + 1) * 128, :]
                            )
                    elif u == 1 and qi == 0:
                        o_projection(0, 0)
                    elif u == 1 and qi == 1:
                        o_projection(0, 1)
                        collective(1, 0)
                    elif u == 1 and qi == 3:
                        # (1,0)'s collective finished during qi2/qi3; emit its
                        # O-projection BEFORE collective(1,1) — collective
                        # completions share one cumulative semaphore, so
                        # anything emitted after cc(1,1) also waits on it.
                        o_projection(1, 0)
                        collective(1, 1)

            o_projection(1, 1)

    nc.compile()
    return nc


def _host_inputs(x, wq, wk, wv, wo):
    import ml_dtypes

    bf = ml_dtypes.bfloat16
    xt = np.ascontiguousarray(x.transpose(0, 2, 1)).astype(bf)
    wot = np.ascontiguousarray(wo.T).astype(bf)

    p = np.arange(128)
    invf = THETA ** (-2.0 * ((p % 64) // 2) / 64.0)
    ang = invf[:, None] * np.arange(S)[None, :]
    cost = np.cos(ang).astype(bf)
    sinmt = (np.sin(ang) * np.where(p % 2 == 0, -1.0, 1.0)[:, None]).astype(bf)

    i = np.arange(128)[:, None]
    j = np.arange(128)[None, :]
    # additive causal triangle: 0 where allowed (j >= i), -1e9 where masked
    trimask = np.where(j >= i, 0.0, -1e9).astype(bf)
    ident = np.eye(128, dtype=np.float32)
    sel = np.zeros((1, 2, 128), np.float32)
    sel[0, 0, 0:64] = 1.0
    sel[0, 1, 64:128] = 1.0

    in_maps = []
    for c in range(NC):
        sl = slice(c * 128, (c + 1) * 128)
        in_maps.append({
            "xt": xt,
            "wqt": np.ascontiguousarray(wq[sl, :].T).astype(bf),
            "wkt": np.ascontiguousarray(wk[sl, :].T).astype(bf),
            "wvt": np.ascontiguousarray(wv[sl, :].T).astype(bf),
            "wot": wot,
            "cost": cost,
            "sinmt": sinmt,
            "trimask": trimask,
            "ident": ident,
            "identb": ident.astype(bf),
            "ones": np.ones((128, 16), np.float32),
            "sel": sel,
        })
    return in_maps


def kernel(x, wq, wk, wv, wo, _trace=False):
    from concourse.bass_utils import run_bass_kernel_spmd

    if "nc" not in _COMPILED:
        _COMPILED["nc"] = _build()
    nc = _COMPILED["nc"]

    in_maps = _host_inputs(
        np.asarray(x, np.float32), np.asarray(wq, np.float32),
        np.asarray(wk, np.float32), np.asarray(wv, np.float32),
        np.asarray(wo, np.float32),
    )
    res = run_bass_kernel_spmd(nc, in_maps, core_ids=list(range(NC)), trace=_trace)
    _COMPILED["last_result"] = res

    y = np.zeros((B, S, D), np.float32)
    for c in range(NC):
        yc = res.results[c]["y"]  # [B, 2, 128, D]
        for u in range(B):
            y[u, 128 * c: 128 * c + 128, :] = yc[u, 0]
            y[u, 1024 + 128 * c: 1024 + 128 * c + 128, :] = yc[u, 1]
    return y


# revision 40
# speedup vs baseline: 1.3658x; 1.0051x over previous
"""Multi-head self-attention (RoPE, causal) Bass kernel for 8 TRN2 NeuronCores.

Sharding: tensor-parallel over heads for QKV+attention (2 heads/core),
chunked AllToAll, then token-parallel O-projection (512 tokens/core).

bf16 data path (fp32 PSUM accumulation + fp32 softmax statistics):
  xt/wq/wk/wv/wo/qT/kT/v/at/aoT/a2a payload are bf16 -> FWL weight loads,
  half DMA + collective bytes. Measured rel err ~6e-3 (gate 2e-2).

Layouts (per core):
  qT/kT:    [128 part = 2 heads x 64 dk, t] bf16 (RoPE'd projections)
  scoresT:  [128 part = k-tile, q free] PSUM f32 (softmax sum via ones-row)
  v_sb:     [128 part = k-tile tokens, 130] bf16 ([v_h0 | ones | v_h1 | ones])
  aoT:      [128 d, 512] bf16 per q-tile, normalized on PSUM unload
  y:        [t, o] f32 token-major final output

Causal masking via -1e9 identity-matmul accumulation (PE, bf16).
Per-unit AllToAll is split in two [NC,128,128] chunks (tokens 0:1024 /
1024:2048) so the last collective + O-projection tail is short; chunk
O-projections are interleaved into the next unit's attention.
"""

import numpy as np

B, S, D, H, DK = 2, 2048, 1024, 16, 64
NC = 8
THETA = 10000.0

_COMPILED = {}


def _build():
    import concourse.bass as bass
    import concourse.tile as tile
    from concourse import bacc, mybir

    f32 = mybir.dt.float32
    f32r = mybir.dt.float32r
    bf16 = mybir.dt.bfloat16
    MUL = mybir.AluOpType.mult
    ADD = mybir.AluOpType.add
    EXP = mybir.ActivationFunctionType.Exp

    nc = bacc.Bacc(num_devices=NC)

    xt_d = nc.dram_tensor("xt", [B, D, S], bf16, kind="ExternalInput")
    wqt_d = nc.dram_tensor("wqt", [D, 128], bf16, kind="ExternalInput")
    wkt_d = nc.dram_tensor("wkt", [D, 128], bf16, kind="ExternalInput")
    wvt_d = nc.dram_tensor("wvt", [D, 128], bf16, kind="ExternalInput")
    wot_d = nc.dram_tensor("wot", [D, D], bf16, kind="ExternalInput")
    cost_d = nc.dram_tensor("cost", [128, S], bf16, kind="ExternalInput")
    sinmt_d = nc.dram_tensor("sinmt", [128, S], bf16, kind="ExternalInput")
    trimask_d = nc.dram_tensor("trimask", [128, 128], bf16, kind="ExternalInput")
    ident_d = nc.dram_tensor("ident", [128, 128], f32, kind="ExternalInput")
    identb_d = nc.dram_tensor("identb", [128, 128], bf16, kind="ExternalInput")
    ones_d = nc.dram_tensor("ones", [128, 16], f32r, kind="ExternalInput")
    y_d = nc.dram_tensor("y", [B, 2, 128, D], f32, kind="ExternalOutput")

    SWAP_MASK = [(i ^ 1) for i in range(32)]

    with tile.TileContext(nc) as tc:
        with (
            tc.tile_pool(name="const", bufs=1) as constp,
            tc.tile_pool(name="xtp", bufs=4) as xtp,
            tc.tile_pool(name="qk", bufs=2) as qkp,
            tc.tile_pool(name="vp", bufs=2) as vp,
            tc.tile_pool(name="attn", bufs=3) as attnp,
            tc.tile_pool(name="ao", bufs=2) as aop,
            tc.tile_pool(name="small", bufs=1) as smallp,
            tc.tile_pool(name="rbp", bufs=2) as rbp,
            tc.tile_pool(name="rtmp", bufs=2) as rtmp,
            tc.tile_pool(name="oproj", bufs=2) as op_,
            tc.tile_pool(name="yp", bufs=2) as yp,
            tc.tile_pool(name="ps", bufs=4, space="PSUM") as psp,
            tc.tile_pool(name="dram", bufs=1, space="DRAM") as dramp,
        ):
            # ---- constant tiles ----
            cost = constp.tile([128, S], bf16)
            sinmt = constp.tile([128, S], bf16)
            trimask = constp.tile([128, 128], bf16)
            ident = constp.tile([128, 128], f32)
            identb = constp.tile([128, 128], bf16)
            ones_sb = constp.tile([128, 16], f32r)
            wq_sb = constp.tile([128, 8, 128], bf16)
            wk_sb = constp.tile([128, 8, 128], bf16)
            wv_sb = constp.tile([128, 8, 128], bf16)
            wo_sb = constp.tile([128, 8, D], bf16)

            # critical path: wq first; wk/wv follow the first xt tile so the
            # q-pass can start as early as possible
            nc.sync.dma_start(wq_sb[:], wqt_d[:, :].rearrange("(dc p) c -> p dc c", dc=8))

            warm_in = dramp.tile([NC, 64], bf16, name="warm_in")
            warm_out = dramp.tile([NC, 64], bf16, name="warm_out")
            nc.gpsimd.collective_compute(
                "AllToAll",
                mybir.AluOpType.bypass,
                replica_groups=[list(range(NC))],
                ins=[warm_in.opt()],
                outs=[warm_out.opt()],
            )
            # chunked a2a: chunk 0 = tokens [0,1024), chunk 1 = [1024,2048)
            a2a_in = [
                [dramp.tile([NC, 128, 128], bf16, name=f"a2ai{u}_{c}") for c in range(2)]
                for u in range(B)
            ]
            a2a_out = [
                [dramp.tile([NC, 128, 128], bf16, name=f"a2ao{u}_{c}") for c in range(2)]
                for u in range(B)
            ]
            recip_dram = dramp.tile([B, 8, 512], f32)

            def collective(u, c):
                nc.gpsimd.collective_compute(
                    "AllToAll",
                    mybir.AluOpType.bypass,
                    replica_groups=[list(range(NC))],
                    ins=[a2a_in[u][c].opt()],
                    outs=[a2a_out[u][c].opt()],
                )

            def o_projection(u, c):
                g = op_.tile([128, 8, 128], bf16, tag="g", name="g")
                nc.sync.dma_start(g[:], a2a_out[u][c].rearrange("s p c -> p s c"))
                y_ps = psp.tile([128, 1024], f32, tag="ps", name="y_ps")
                for os_ in range(2):
                    for dc in range(8):
                        nc.tensor.matmul(
                            y_ps[:, os_ * 512:(os_ + 1) * 512],
                            g[:, dc, :],
                            wo_sb[:, dc, os_ * 512:(os_ + 1) * 512],
                            start=(dc == 0), stop=(dc == 7),
                            skip_group_check=True,
                        )
                y_sb = yp.tile([128, D], f32, tag="y", name="y_sb")
                nc.vector.tensor_copy(out=y_sb[:], in_=y_ps[:])
                nc.sync.dma_start(y_d[u, c], y_sb[:])

            def load_xt(u, first):
                tiles = []
                for tt in range(4):
                    ts = slice(tt * 512, (tt + 1) * 512)
                    xt_sb = xtp.tile([128, 8, 512], bf16, tag="xt", name="xt_sb")
                    src = xt_d[u, :, ts].rearrange("(dc p) s -> p dc s", dc=8)
                    nc.sync.dma_start(xt_sb[:, 0:4, :], src[:, 0:4, :])
                    nc.sync.dma_start(xt_sb[:, 4:8, :], src[:, 4:8, :])
                    if first and tt == 0:
                        nc.sync.dma_start(
                            wk_sb[:], wkt_d[:, :].rearrange("(dc p) c -> p dc c", dc=8))
                        nc.sync.dma_start(
                            wv_sb[:], wvt_d[:, :].rearrange("(dc p) c -> p dc c", dc=8))
                        # non-critical consts after the first xt tile
                        nc.gpsimd.dma_start(cost[:, 0:1024], cost_d[:, 0:1024])
                        nc.gpsimd.dma_start(sinmt[:, 0:1024], sinmt_d[:, 0:1024])
                        nc.gpsimd.dma_start(ident[:], ident_d[:])
                        nc.gpsimd.dma_start(identb[:], identb_d[:])
                        nc.gpsimd.dma_start(ones_sb[:], ones_d[:])
                        nc.gpsimd.dma_start(trimask[:], trimask_d[:])
                        nc.gpsimd.dma_start(cost[:, 1024:2048], cost_d[:, 1024:2048])
                        nc.gpsimd.dma_start(sinmt[:, 1024:2048], sinmt_d[:, 1024:2048])
                    tiles.append(xt_sb)
                return tiles

            xts = {0: load_xt(0, True)}

            for u in range(B):
                # ================= projections + RoPE =================
                qT = qkp.tile([128, S], bf16, tag="qT", name="qT")
                kT = qkp.tile([128, S], bf16, tag="kT", name="kT")
                v_sb = vp.tile([128, 16, 130], f32r, tag="v", name="v_sb")

                for tt in range(4):
                    ts = slice(tt * 512, (tt + 1) * 512)
                    xt_sb = xts[u][tt]
                    qk_ps = psp.tile([128, 1024], f32, tag="ps", name="qk_ps")
                    v_ps = psp.tile([128, 1024], f32, tag="ps", name="v_ps")
                    for w_sb, dst in ((wq_sb, qk_ps[:, 0:512]),
                                      (wk_sb, qk_ps[:, 512:1024]),
                                      (wv_sb, v_ps[:, 0:512])):
                        for dc in range(8):
                            nc.tensor.matmul(dst, w_sb[:, dc, :], xt_sb[:, dc, :],
                                             start=(dc == 0), stop=(dc == 7))

                    # RoPE: dst = q*cos + pairswap(q)*sinm  (bf16 out)
                    for src_, dst in ((qk_ps[:, 0:512], qT), (qk_ps[:, 512:1024], kT)):
                        qs = rtmp.tile([128, 512], f32, tag="qs", name="qs")
                        t2 = rtmp.tile([128, 512], bf16, tag="t2", name="t2")
                        nc.vector.stream_shuffle(qs[:], src_, SWAP_MASK)
                        nc.vector.tensor_tensor(out=dst[:, ts], in0=src_, in1=cost[:, ts], op=MUL)
                        nc.vector.tensor_tensor(out=t2[:], in0=qs[:], in1=sinmt[:, ts], op=MUL)
                        nc.vector.tensor_tensor(out=dst[:, ts], in0=dst[:, ts], in1=t2[:], op=ADD)

                    # v -> token-major via PE transpose; ones columns appended
                    vtmp = rtmp.tile([128, 512], f32, tag="vtmp", name="vtmp")
                    nc.vector.tensor_copy(out=vtmp[:], in_=v_ps[:, 0:512])
                    for s4 in range(4):
                        kt = tt * 4 + s4
                        tr = v_ps[:, 512 + s4 * 128: 512 + (s4 + 1) * 128]
                        nc.tensor.transpose(tr, vtmp[:, s4 * 128:(s4 + 1) * 128], ident[:])
                        dst = v_sb[:, kt, :].rearrange("p (u c) -> p u c", u=2)[:, :, 0:64]
                        src_ = tr.rearrange("p (u c) -> p u c", u=2)
                        nc.vector.tensor_copy(out=dst, in_=src_)
                    nc.vector.tensor_copy(out=v_sb[:, tt * 4:(tt + 1) * 4, 64:65],
                                          in_=ones_sb[:, tt * 4:(tt + 1) * 4])
                    nc.vector.tensor_copy(out=v_sb[:, tt * 4:(tt + 1) * 4, 129:130],
                                          in_=ones_sb[:, tt * 4:(tt + 1) * 4])

                if u == 0:
                    # prefetch unit 1's activations during unit 0's attention
                    xts[1] = load_xt(1, False)

                # ================= attention =================
                for qi in range(4):
                    qsl = slice(qi * 512, (qi + 1) * 512)
                    outT = psp.tile([128, 1024], f32, tag="ps", name="outT")
                    n_kt = 4 * qi + 4
                    # software pipeline (depth 2): AV for tile kt is emitted
                    # after scores/exp of tile kt+2 so the PE isn't stalled
                    # on the ACT engine's exp
                    pend = []  # (at, kt, q0)

                    def flush_av(last):
                        at_, kt_, q0 = pend.pop(0)
                        for h in (0, 1):
                            nc.tensor.matmul(
                                outT[0:65, h * 512 + q0:(h + 1) * 512],
                                v_sb[:, kt_, :].rearrange("p (u c) -> p u c", u=2)[:, h, :],
                                at_[:, h * 512 + q0:(h + 1) * 512],
                                start=(kt_ == 0), stop=last,
                                skip_group_check=True,
                            )

                    for kt in range(n_kt):
                        ksl = slice(kt * 128, (kt + 1) * 128)
                        rel = kt - 4 * qi  # >=0 on the diagonal block
                        q0 = 128 * rel if rel > 0 else 0
                        sc = psp.tile([128, 1024], f32, tag="ps", name="sc")
                        at = attnp.tile([128, 1024], f32r, tag="at", name="at")
                        for h in (0, 1):
                            hp = slice(h * 64, (h + 1) * 64)
                            nc.tensor.matmul(
                                sc[:, h * 512 + q0:(h + 1) * 512],
                                kT[hp, ksl],
                                qT[hp, qsl][:, q0:512],
                                start=True, stop=(rel < 0),
                                skip_group_check=True,
                            )
                        if rel >= 0:
                            # causal triangle at q columns [q0, q0+128)
                            for h in (0, 1):
                                nc.tensor.matmul(
                                    sc[:, h * 512 + q0: h * 512 + q0 + 128],
                                    identb[:],
                                    trimask[:],
                                    start=False, stop=True,
                                    skip_group_check=True,
                                )
                        if len(pend) >= 2:
                            flush_av(False)
                        if q0 == 0:
                            nc.scalar.activation(out=at[:], in_=sc[:], func=EXP, scale=0.125)
                        else:
                            scs = sc.rearrange("p (h q) -> p h q", h=2)[:, :, q0:512]
                            ats = at.rearrange("p (h q) -> p h q", h=2)[:, :, q0:512]
                            nc.scalar.activation(out=ats, in_=scs, func=EXP, scale=0.125)
                        pend.append((at, kt, q0))
                    while len(pend) > 1:
                        flush_av(False)
                    flush_av(True)

                    # unload + normalize + ship this q-tile.  1/den per head is
                    # broadcast across 64 partitions by a K=2 selector matmul
                    # into the (now free) last score tile's PSUM region.
                    aoT = aop.tile([128, 512], bf16, tag="aoT", name="aoT")
                    rb = rbp.tile([128, 512], f32, tag="rb", name="rb")
                    for h in (0, 1):
                        nc.vector.tensor_copy(
                            out=aoT[h * 64:(h + 1) * 64, :],
                            in_=outT[0:64, h * 512:(h + 1) * 512],
                        )
                        dent = smallp.tile([1, 512], f32, tag=f"dent{h}",
                                           name="dent", bufs=2)
                        rc = smallp.tile([1, 512], f32, tag=f"rc{h}",
                                         name="rc", bufs=2)
                        nc.vector.tensor_copy(
                            out=dent[0:1, :], in_=outT[64:65, h * 512:(h + 1) * 512]
                        )
                        nc.vector.reciprocal_approx_fast(out=rc[0:1, :], in_=dent[0:1, :])
                        nc.sync.dma_start(
                            recip_dram[u, h * 4 + qi: h * 4 + qi + 1, :], rc[0:1, :]
                        )
                        nc.gpsimd.dma_start(
                            rb[h * 64:(h + 1) * 64, :],
                            recip_dram[u, h * 4 + qi: h * 4 + qi + 1, :].to_broadcast([64, 512]),
                        )
                    nc.vector.tensor_tensor(out=aoT[:], in0=aoT[:], in1=rb[:], op=MUL)
                    # ship: chunk c = qi//2, slots 4*(qi%2) .. +4
                    c = qi // 2
                    j0 = 4 * (qi % 2)
                    nc.sync.dma_start(
                        a2a_in[u][c][j0:j0 + 4].rearrange("s p c -> p s c"),
                        aoT[:].rearrange("p (s c) -> p s c", s=4),
                    )
                    if u == 0 and qi == 1:
                        collective(0, 0)
                    elif u == 0 and qi == 3:
                        collective(0, 1)
                        # O-projection weights: off the startup critical path
                        for dc in range(8):
                            nc.gpsimd.dma_start(
                                wo_sb[:, dc, :], wot_d[dc * 128:(dc + 1) * 128, :]
                            )
                    elif u == 1 and qi == 0:
                        o_projection(0, 0)
                    elif u == 1 and qi == 1:
                        o_projection(0, 1)
                        collective(1, 0)
                    elif u == 1 and qi == 2:
                        # (1,0)'s collective finished during qi2; emitting its
                        # O-projection here (before cc(1,1) exists) keeps its
                        # semaphore threshold at A1 and shortens the tail to a
                        # single chunk after the last collective.
                        o_projection(1, 0)
                    elif u == 1 and qi == 3:
                        collective(1, 1)

            o_projection(1, 1)

    nc.compile()
    return nc


def _host_inputs(x, wq, wk, wv, wo):
    import ml_dtypes

    bf = ml_dtypes.bfloat16
    xt = np.ascontiguousarray(x.transpose(0, 2, 1)).astype(bf)
    wot = np.ascontiguousarray(wo.T).astype(bf)

    p = np.arange(128)
    invf = THETA ** (-2.0 * ((p % 64) // 2) / 64.0)
    ang = invf[:, None] * np.arange(S)[None, :]
    cost = np.cos(ang).astype(bf)
    sinmt = (np.sin(ang) * np.where(p % 2 == 0, -1.0, 1.0)[:, None]).astype(bf)

    i = np.arange(128)[:, None]
    j = np.arange(128)[None, :]
    # additive causal triangle: 0 where allowed (j >= i), -1e9 where masked
    trimask = np.where(j >= i, 0.0, -1e9).astype(bf)
    ident = np.eye(128, dtype=np.float32)

    in_maps = []
    for c in range(NC):
        sl = slice(c * 128, (c + 1) * 128)
        in_maps.append({
            "xt": xt,
            "wqt": np.ascontiguousarray(wq[sl, :].T).astype(bf),
            "wkt": np.ascontiguousarray(wk[sl, :].T).astype(bf),
            "wvt": np.ascontiguousarray(wv[sl, :].T).astype(bf),
            "wot": wot,
            "cost": cost,
            "sinmt": sinmt,
            "trimask": trimask,
            "ident": ident,
            "identb": ident.astype(bf),
            "ones": np.ones((128, 16), np.float32),
        })
    return in_maps


def kernel(x, wq, wk, wv, wo, _trace=False):
    from concourse.bass_utils import run_bass_kernel_spmd

    if "nc" not in _COMPILED:
        _COMPILED["nc"] = _build()
    nc = _COMPILED["nc"]

    in_maps = _host_inputs(
        np.asarray(x, np.float32), np.asarray(wq, np.float32),
        np.asarray(wk, np.float32), np.asarray(wv, np.float32),
        np.asarray(wo, np.float32),
    )
    res = run_bass_kernel_spmd(nc, in_maps, core_ids=list(range(NC)), trace=_trace)
    _COMPILED["last_result"] = res

    y = np.zeros((B, S, D), np.float32)
    for c in range(NC):
        yc = res.results[c]["y"]  # [B, 2, 128, D]
        for u in range(B):
            y[u, 128 * c: 128 * c + 128, :] = yc[u, 0]
            y[u, 1024 + 128 * c: 1024 + 128 * c + 128, :] = yc[u, 1]
    return y


# revision 42
# speedup vs baseline: 1.4501x; 1.0617x over previous
"""Multi-head self-attention (RoPE, causal) Bass kernel for 8 TRN2 NeuronCores.

Sharding: tensor-parallel over heads for QKV+attention (2 heads/core),
chunked AllToAll, then token-parallel O-projection (512 tokens/core).

bf16 data path (fp32 PSUM accumulation + fp32 softmax statistics):
  xt/wq/wk/wv/wo/qT/kT/v/at/aoT/a2a payload are bf16 -> FWL weight loads,
  half DMA + collective bytes. Measured rel err ~6e-3 (gate 2e-2).

Layouts (per core):
  qT/kT:    [128 part = 2 heads x 64 dk, t] bf16 (RoPE'd projections)
  scoresT:  [128 part = k-tile, q free] PSUM f32 (softmax sum via ones-row)
  v_sb:     [128 part = k-tile tokens, 130] bf16 ([v_h0 | ones | v_h1 | ones])
  aoT:      [128 d, 512] bf16 per q-tile, normalized on PSUM unload
  y:        [t, o] f32 token-major final output

Causal masking via -1e9 identity-matmul accumulation (PE, bf16).
Per-unit AllToAll is split in two [NC,128,128] chunks (tokens 0:1024 /
1024:2048) so the last collective + O-projection tail is short; chunk
O-projections are interleaved into the next unit's attention.
"""

import numpy as np

B, S, D, H, DK = 2, 2048, 1024, 16, 64
NC = 8
THETA = 10000.0

_COMPILED = {}


def _build():
    import concourse.bass as bass
    import concourse.tile as tile
    from concourse import bacc, mybir

    f32 = mybir.dt.float32
    f32r = mybir.dt.float32r
    bf16 = mybir.dt.bfloat16
    MUL = mybir.AluOpType.mult
    ADD = mybir.AluOpType.add
    EXP = mybir.ActivationFunctionType.Exp

    nc = bacc.Bacc(num_devices=NC)

    xt_d = nc.dram_tensor("xt", [B, D, S], bf16, kind="ExternalInput")
    wqt_d = nc.dram_tensor("wqt", [D, 128], bf16, kind="ExternalInput")
    wkt_d = nc.dram_tensor("wkt", [D, 128], bf16, kind="ExternalInput")
    wvt_d = nc.dram_tensor("wvt", [D, 128], bf16, kind="ExternalInput")
    wot_d = nc.dram_tensor("wot", [D, D], bf16, kind="ExternalInput")
    cost_d = nc.dram_tensor("cost", [128, S], bf16, kind="ExternalInput")
    sinmt_d = nc.dram_tensor("sinmt", [128, S], bf16, kind="ExternalInput")
    trimask_d = nc.dram_tensor("trimask", [128, 128], bf16, kind="ExternalInput")
    ident_d = nc.dram_tensor("ident", [128, 128], f32, kind="ExternalInput")
    identb_d = nc.dram_tensor("identb", [128, 128], bf16, kind="ExternalInput")
    ones_d = nc.dram_tensor("ones", [128, 16], f32r, kind="ExternalInput")
    y_d = nc.dram_tensor("y", [B, 2, 128, D], f32, kind="ExternalOutput")

    SWAP_MASK = [(i ^ 1) for i in range(32)]

    with tile.TileContext(nc) as tc:
        with (
            tc.tile_pool(name="const", bufs=1) as constp,
            tc.tile_pool(name="xtp", bufs=4) as xtp,
            tc.tile_pool(name="qk", bufs=2) as qkp,
            tc.tile_pool(name="vp", bufs=2) as vp,
            tc.tile_pool(name="attn", bufs=3) as attnp,
            tc.tile_pool(name="ao", bufs=2) as aop,
            tc.tile_pool(name="small", bufs=1) as smallp,
            tc.tile_pool(name="rbp", bufs=2) as rbp,
            tc.tile_pool(name="rtmp", bufs=2) as rtmp,
            tc.tile_pool(name="oproj", bufs=2) as op_,
            tc.tile_pool(name="yp", bufs=2) as yp,
            tc.tile_pool(name="ps", bufs=4, space="PSUM") as psp,
            tc.tile_pool(name="dram", bufs=1, space="DRAM") as dramp,
        ):
            # ---- constant tiles ----
            cost = constp.tile([128, S], bf16)
            sinmt = constp.tile([128, S], bf16)
            trimask = constp.tile([128, 128], bf16)
            ident = constp.tile([128, 128], f32)
            identb = constp.tile([128, 128], bf16)
            ones_sb = constp.tile([128, 16], f32r)
            wq_sb = constp.tile([128, 8, 128], bf16)
            wk_sb = constp.tile([128, 8, 128], bf16)
            wv_sb = constp.tile([128, 8, 128], bf16)
            wo_sb = constp.tile([128, 8, D], bf16)

            # critical path: wq first; wk/wv follow the first xt tile so the
            # q-pass can start as early as possible
            nc.sync.dma_start(wq_sb[:], wqt_d[:, :].rearrange("(dc p) c -> p dc c", dc=8))

            warm_in = dramp.tile([NC, 64], bf16, name="warm_in")
            warm_out = dramp.tile([NC, 64], bf16, name="warm_out")
            nc.gpsimd.collective_compute(
                "AllToAll",
                mybir.AluOpType.bypass,
                replica_groups=[list(range(NC))],
                ins=[warm_in.opt()],
                outs=[warm_out.opt()],
            )
            # chunked a2a: chunk 0 = tokens [0,1024), chunk 1 = [1024,2048)
            a2a_in = [
                [dramp.tile([NC, 128, 128], bf16, name=f"a2ai{u}_{c}") for c in range(2)]
                for u in range(B)
            ]
            a2a_out = [
                [dramp.tile([NC, 128, 128], bf16, name=f"a2ao{u}_{c}") for c in range(2)]
                for u in range(B)
            ]
            recip_dram = dramp.tile([B, 8, 512], f32)

            def collective(u, c):
                nc.gpsimd.collective_compute(
                    "AllToAll",
                    mybir.AluOpType.bypass,
                    replica_groups=[list(range(NC))],
                    ins=[a2a_in[u][c].opt()],
                    outs=[a2a_out[u][c].opt()],
                )

            def o_projection(u, c):
                g = op_.tile([128, 8, 128], bf16, tag="g", name="g")
                nc.gpsimd.dma_start(g[:], a2a_out[u][c].rearrange("s p c -> p s c"))
                y_ps = psp.tile([128, 1024], f32, tag="ps", name="y_ps")
                for os_ in range(2):
                    for dc in range(8):
                        nc.tensor.matmul(
                            y_ps[:, os_ * 512:(os_ + 1) * 512],
                            g[:, dc, :],
                            wo_sb[:, dc, os_ * 512:(os_ + 1) * 512],
                            start=(dc == 0), stop=(dc == 7),
                            skip_group_check=True,
                        )
                y_sb = yp.tile([128, D], f32, tag="y", name="y_sb")
                nc.vector.tensor_copy(out=y_sb[:], in_=y_ps[:])
                nc.sync.dma_start(y_d[u, c], y_sb[:])

            def load_xt(u, first):
                tiles = []
                for tt in range(4):
                    ts = slice(tt * 512, (tt + 1) * 512)
                    xt_sb = xtp.tile([128, 8, 512], bf16, tag="xt", name="xt_sb")
                    src = xt_d[u, :, ts].rearrange("(dc p) s -> p dc s", dc=8)
                    nc.sync.dma_start(xt_sb[:, 0:4, :], src[:, 0:4, :])
                    nc.sync.dma_start(xt_sb[:, 4:8, :], src[:, 4:8, :])
                    if first and tt == 0:
                        nc.sync.dma_start(
                            wk_sb[:], wkt_d[:, :].rearrange("(dc p) c -> p dc c", dc=8))
                        nc.sync.dma_start(
                            wv_sb[:], wvt_d[:, :].rearrange("(dc p) c -> p dc c", dc=8))
                        # non-critical consts after the first xt tile
                        nc.gpsimd.dma_start(cost[:, 0:1024], cost_d[:, 0:1024])
                        nc.gpsimd.dma_start(sinmt[:, 0:1024], sinmt_d[:, 0:1024])
                        nc.gpsimd.dma_start(ident[:], ident_d[:])
                        nc.gpsimd.dma_start(identb[:], identb_d[:])
                        nc.gpsimd.dma_start(ones_sb[:], ones_d[:])
                        nc.gpsimd.dma_start(trimask[:], trimask_d[:])
                        nc.gpsimd.dma_start(cost[:, 1024:2048], cost_d[:, 1024:2048])
                        nc.gpsimd.dma_start(sinmt[:, 1024:2048], sinmt_d[:, 1024:2048])
                    tiles.append(xt_sb)
                return tiles

            xts = {0: load_xt(0, True)}

            for u in range(B):
                # ================= projections + RoPE =================
                qT = qkp.tile([128, S], bf16, tag="qT", name="qT")
                kT = qkp.tile([128, S], bf16, tag="kT", name="kT")
                v_sb = vp.tile([128, 16, 130], f32r, tag="v", name="v_sb")

                for tt in range(4):
                    ts = slice(tt * 512, (tt + 1) * 512)
                    xt_sb = xts[u][tt]
                    qk_ps = psp.tile([128, 1024], f32, tag="ps", name="qk_ps")
                    v_ps = psp.tile([128, 1024], f32, tag="ps", name="v_ps")
                    for w_sb, dst in ((wq_sb, qk_ps[:, 0:512]),
                                      (wk_sb, qk_ps[:, 512:1024]),
                                      (wv_sb, v_ps[:, 0:512])):
                        for dc in range(8):
                            nc.tensor.matmul(dst, w_sb[:, dc, :], xt_sb[:, dc, :],
                                             start=(dc == 0), stop=(dc == 7))

                    # RoPE: dst = q*cos + pairswap(q)*sinm  (bf16 out)
                    for src_, dst in ((qk_ps[:, 0:512], qT), (qk_ps[:, 512:1024], kT)):
                        qs = rtmp.tile([128, 512], f32, tag="qs", name="qs")
                        t2 = rtmp.tile([128, 512], bf16, tag="t2", name="t2")
                        nc.vector.stream_shuffle(qs[:], src_, SWAP_MASK)
                        nc.vector.tensor_tensor(out=dst[:, ts], in0=src_, in1=cost[:, ts], op=MUL)
                        nc.vector.tensor_tensor(out=t2[:], in0=qs[:], in1=sinmt[:, ts], op=MUL)
                        nc.vector.tensor_tensor(out=dst[:, ts], in0=dst[:, ts], in1=t2[:], op=ADD)

                    # v -> token-major via PE transpose; ones columns appended
                    vtmp = rtmp.tile([128, 512], f32, tag="vtmp", name="vtmp")
                    nc.vector.tensor_copy(out=vtmp[:], in_=v_ps[:, 0:512])
                    for s4 in range(4):
                        kt = tt * 4 + s4
                        tr = v_ps[:, 512 + s4 * 128: 512 + (s4 + 1) * 128]
                        nc.tensor.transpose(tr, vtmp[:, s4 * 128:(s4 + 1) * 128], ident[:])
                        dst = v_sb[:, kt, :].rearrange("p (u c) -> p u c", u=2)[:, :, 0:64]
                        src_ = tr.rearrange("p (u c) -> p u c", u=2)
                        nc.vector.tensor_copy(out=dst, in_=src_)
                    nc.vector.tensor_copy(out=v_sb[:, tt * 4:(tt + 1) * 4, 64:65],
                                          in_=ones_sb[:, tt * 4:(tt + 1) * 4])
                    nc.vector.tensor_copy(out=v_sb[:, tt * 4:(tt + 1) * 4, 129:130],
                                          in_=ones_sb[:, tt * 4:(tt + 1) * 4])

                if u == 0:
                    # prefetch unit 1's activations during unit 0's attention
                    xts[1] = load_xt(1, False)

                # ================= attention =================
                for qi in range(4):
                    qsl = slice(qi * 512, (qi + 1) * 512)
                    outT = psp.tile([128, 1024], f32, tag="ps", name="outT")
                    n_kt = 4 * qi + 4
                    # software pipeline (depth 2): AV for tile kt is emitted
                    # after scores/exp of tile kt+2 so the PE isn't stalled
                    # on the ACT engine's exp
                    pend = []  # (at, kt, q0)

                    def flush_av(last):
                        at_, kt_, q0 = pend.pop(0)
                        for h in (0, 1):
                            nc.tensor.matmul(
                                outT[0:65, h * 512 + q0:(h + 1) * 512],
                                v_sb[:, kt_, :].rearrange("p (u c) -> p u c", u=2)[:, h, :],
                                at_[:, h * 512 + q0:(h + 1) * 512],
                                start=(kt_ == 0), stop=last,
                                skip_group_check=True,
                            )

                    for kt in range(n_kt):
                        ksl = slice(kt * 128, (kt + 1) * 128)
                        rel = kt - 4 * qi  # >=0 on the diagonal block
                        q0 = 128 * rel if rel > 0 else 0
                        sc = psp.tile([128, 1024], f32, tag="ps", name="sc")
                        at = attnp.tile([128, 1024], f32r, tag="at", name="at")
                        for h in (0, 1):
                            hp = slice(h * 64, (h + 1) * 64)
                            nc.tensor.matmul(
                                sc[:, h * 512 + q0:(h + 1) * 512],
                                kT[hp, ksl],
                                qT[hp, qsl][:, q0:512],
                                start=True, stop=(rel < 0),
                                skip_group_check=True,
                            )
                        if rel >= 0:
                            # causal triangle at q columns [q0, q0+128)
                            for h in (0, 1):
                                nc.tensor.matmul(
                                    sc[:, h * 512 + q0: h * 512 + q0 + 128],
                                    identb[:],
                                    trimask[:],
                                    start=False, stop=True,
                                    skip_group_check=True,
                                )
                        if len(pend) >= 2:
                            flush_av(False)
                        if q0 == 0:
                            nc.scalar.activation(out=at[:], in_=sc[:], func=EXP, scale=0.125)
                        else:
                            scs = sc.rearrange("p (h q) -> p h q", h=2)[:, :, q0:512]
                            ats = at.rearrange("p (h q) -> p h q", h=2)[:, :, q0:512]
                            nc.scalar.activation(out=ats, in_=scs, func=EXP, scale=0.125)
                        pend.append((at, kt, q0))
                    while len(pend) > 1:
                        flush_av(False)
                    flush_av(True)

                    # unload + normalize + ship this q-tile.  1/den per head is
                    # broadcast across 64 partitions by a K=2 selector matmul
                    # into the (now free) last score tile's PSUM region.
                    aoT = aop.tile([128, 512], bf16, tag="aoT", name="aoT")
                    rb = rbp.tile([128, 512], f32, tag="rb", name="rb")
                    for h in (0, 1):
                        nc.vector.tensor_copy(
                            out=aoT[h * 64:(h + 1) * 64, :],
                            in_=outT[0:64, h * 512:(h + 1) * 512],
                        )
                        dent = smallp.tile([1, 512], f32, tag=f"dent{h}",
                                           name="dent", bufs=2)
                        rc = smallp.tile([1, 512], f32, tag=f"rc{h}",
                                         name="rc", bufs=2)
                        nc.vector.tensor_copy(
                            out=dent[0:1, :], in_=outT[64:65, h * 512:(h + 1) * 512]
                        )
                        nc.vector.reciprocal_approx_fast(out=rc[0:1, :], in_=dent[0:1, :])
                        nc.sync.dma_start(
                            recip_dram[u, h * 4 + qi: h * 4 + qi + 1, :], rc[0:1, :]
                        )
                        nc.gpsimd.dma_start(
                            rb[h * 64:(h + 1) * 64, :],
                            recip_dram[u, h * 4 + qi: h * 4 + qi + 1, :].to_broadcast([64, 512]),
                        )
                    nc.vector.tensor_tensor(out=aoT[:], in0=aoT[:], in1=rb[:], op=MUL)
                    # ship: chunk c = qi//2, slots 4*(qi%2) .. +4
                    c = qi // 2
                    j0 = 4 * (qi % 2)
                    nc.sync.dma_start(
                        a2a_in[u][c][j0:j0 + 4].rearrange("s p c -> p s c"),
                        aoT[:].rearrange("p (s c) -> p s c", s=4),
                    )
                    if u == 0 and qi == 1:
                        collective(0, 0)
                    elif u == 0 and qi == 3:
                        collective(0, 1)
                        # O-projection weights: off the startup critical path
                        for dc in range(8):
                            nc.gpsimd.dma_start(
                                wo_sb[:, dc, :], wot_d[dc * 128:(dc + 1) * 128, :]
                            )
                    elif u == 1 and qi == 0:
                        o_projection(0, 0)
                    elif u == 1 and qi == 1:
                        o_projection(0, 1)
                        collective(1, 0)
                    elif u == 1 and qi == 3:
                        # collective-dependent work must come AFTER all
                        # independent attention work in the in-order engine
                        # queues: this core typically runs ahead of the
                        # rendezvous, so an early-emitted O-projection would
                        # stall the PE before qi3.
                        o_projection(1, 0)
                        collective(1, 1)

            o_projection(1, 1)

    nc.compile()
    return nc


def _host_inputs(x, wq, wk, wv, wo):
    import ml_dtypes

    bf = ml_dtypes.bfloat16
    xt = np.ascontiguousarray(x.transpose(0, 2, 1)).astype(bf)
    wot = np.ascontiguousarray(wo.T).astype(bf)

    p = np.arange(128)
    invf = THETA ** (-2.0 * ((p % 64) // 2) / 64.0)
    ang = invf[:, None] * np.arange(S)[None, :]
    cost = np.cos(ang).astype(bf)
    sinmt = (np.sin(ang) * np.where(p % 2 == 0, -1.0, 1.0)[:, None]).astype(bf)

    i = np.arange(128)[:, None]
    j = np.arange(128)[None, :]
    # additive causal triangle: 0 where allowed (j >= i), -1e9 where masked
    trimask = np.where(j >= i, 0.0, -1e9).astype(bf)
    ident = np.eye(128, dtype=np.float32)

    in_maps = []
    for c in range(NC):
        sl = slice(c * 128, (c + 1) * 128)
        in_maps.append({
            "xt": xt,
            "wqt": np.ascontiguousarray(wq[sl, :].T).astype(bf),
            "wkt": np.ascontiguousarray(wk[sl, :].T).astype(bf),
            "wvt": np.ascontiguousarray(wv[sl, :].T).astype(bf),
            "wot": wot,
            "cost": cost,
            "sinmt": sinmt,
            "trimask": trimask,
            "ident": ident,
            "identb": ident.astype(bf),
            "ones": np.ones((128, 16), np.float32),
        })
    return in_maps


def kernel(x, wq, wk, wv, wo, _trace=False):
    from concourse.bass_utils import run_bass_kernel_spmd

    if "nc" not in _COMPILED:
        _COMPILED["nc"] = _build()
    nc = _COMPILED["nc"]

    in_maps = _host_inputs(
        np.asarray(x, np.float32), np.asarray(wq, np.float32),
        np.asarray(wk, np.float32), np.asarray(wv, np.float32),
        np.asarray(wo, np.float32),
    )
    res = run_bass_kernel_spmd(nc, in_maps, core_ids=list(range(NC)), trace=_trace)
    _COMPILED["last_result"] = res

    y = np.zeros((B, S, D), np.float32)
    for c in range(NC):
        yc = res.results[c]["y"]  # [B, 2, 128, D]
        for u in range(B):
            y[u, 128 * c: 128 * c + 128, :] = yc[u, 0]
            y[u, 1024 + 128 * c: 1024 + 128 * c + 128, :] = yc[u, 1]
    return y


# revision 43
# speedup vs baseline: 1.6121x; 1.1117x over previous
"""Multi-head self-attention (RoPE, causal) Bass kernel for 8 TRN2 NeuronCores.

Sharding: tensor-parallel over heads for QKV+attention (2 heads/core),
chunked AllToAll, then token-parallel O-projection (512 tokens/core).

bf16 data path (fp32 PSUM accumulation + fp32 softmax statistics):
  xt/wq/wk/wv/wo/qT/kT/v/at/aoT/a2a payload are bf16 -> FWL weight loads,
  half DMA + collective bytes. Measured rel err ~6e-3 (gate 2e-2).

Layouts (per core):
  qT/kT:    [128 part = 2 heads x 64 dk, t] bf16 (RoPE'd projections)
  scoresT:  [128 part = k-tile, q free] PSUM f32 (softmax sum via ones-row)
  v_sb:     [128 part = k-tile tokens, 130] bf16 ([v_h0 | ones | v_h1 | ones])
  aoT:      [128 d, 512] bf16 per q-tile, normalized on PSUM unload
  y:        [t, o] f32 token-major final output

Causal masking via -1e9 identity-matmul accumulation (PE, bf16).
Per-unit AllToAll is split in two [NC,128,128] chunks (tokens 0:1024 /
1024:2048) so the last collective + O-projection tail is short; chunk
O-projections are interleaved into the next unit's attention.
"""

import numpy as np

B, S, D, H, DK = 2, 2048, 1024, 16, 64
NC = 8
THETA = 10000.0

_COMPILED = {}


def _build():
    import concourse.bass as bass
    import concourse.tile as tile
    from concourse import bacc, mybir

    f32 = mybir.dt.float32
    f32r = mybir.dt.float32r
    bf16 = mybir.dt.bfloat16
    MUL = mybir.AluOpType.mult
    ADD = mybir.AluOpType.add
    EXP = mybir.ActivationFunctionType.Exp

    nc = bacc.Bacc(num_devices=NC)

    xt_d = nc.dram_tensor("xt", [B, D, S], bf16, kind="ExternalInput")
    wqt_d = nc.dram_tensor("wqt", [D, 128], bf16, kind="ExternalInput")
    wkt_d = nc.dram_tensor("wkt", [D, 128], bf16, kind="ExternalInput")
    wvt_d = nc.dram_tensor("wvt", [D, 128], bf16, kind="ExternalInput")
    wot_d = nc.dram_tensor("wot", [D, D], bf16, kind="ExternalInput")
    cost_d = nc.dram_tensor("cost", [128, S], bf16, kind="ExternalInput")
    sinmt_d = nc.dram_tensor("sinmt", [128, S], bf16, kind="ExternalInput")
    trimask_d = nc.dram_tensor("trimask", [128, 128], bf16, kind="ExternalInput")
    ident_d = nc.dram_tensor("ident", [128, 128], f32, kind="ExternalInput")
    identb_d = nc.dram_tensor("identb", [128, 128], bf16, kind="ExternalInput")
    ones_d = nc.dram_tensor("ones", [128, 16], f32r, kind="ExternalInput")
    y_d = nc.dram_tensor("y", [B, 2, 128, D], f32, kind="ExternalOutput")

    SWAP_MASK = [(i ^ 1) for i in range(32)]

    with tile.TileContext(nc) as tc:
        with (
            tc.tile_pool(name="const", bufs=1) as constp,
            tc.tile_pool(name="xtp", bufs=4) as xtp,
            tc.tile_pool(name="qk", bufs=2) as qkp,
            tc.tile_pool(name="vp", bufs=2) as vp,
            tc.tile_pool(name="attn", bufs=3) as attnp,
            tc.tile_pool(name="ao", bufs=2) as aop,
            tc.tile_pool(name="small", bufs=1) as smallp,
            tc.tile_pool(name="rbp", bufs=2) as rbp,
            tc.tile_pool(name="rtmp", bufs=2) as rtmp,
            tc.tile_pool(name="oproj", bufs=2) as op_,
            tc.tile_pool(name="yp", bufs=2) as yp,
            tc.tile_pool(name="ps", bufs=4, space="PSUM") as psp,
            tc.tile_pool(name="dram", bufs=1, space="DRAM") as dramp,
        ):
            # ---- constant tiles ----
            cost = constp.tile([128, S], bf16)
            sinmt = constp.tile([128, S], bf16)
            trimask = constp.tile([128, 128], bf16)
            ident = constp.tile([128, 128], f32)
            identb = constp.tile([128, 128], bf16)
            ones_sb = constp.tile([128, 16], f32r)
            wq_sb = constp.tile([128, 8, 128], bf16)
            wk_sb = constp.tile([128, 8, 128], bf16)
            wv_sb = constp.tile([128, 8, 128], bf16)
            wo_sb = constp.tile([128, 8, D], bf16)

            # critical path: wq first; wk/wv follow the first xt tile so the
            # q-pass can start as early as possible
            nc.sync.dma_start(wq_sb[:], wqt_d[:, :].rearrange("(dc p) c -> p dc c", dc=8))

            warm_in = dramp.tile([NC, 64], bf16, name="warm_in")
            warm_out = dramp.tile([NC, 64], bf16, name="warm_out")
            nc.gpsimd.collective_compute(
                "AllToAll",
                mybir.AluOpType.bypass,
                replica_groups=[list(range(NC))],
                ins=[warm_in.opt()],
                outs=[warm_out.opt()],
            )
            # chunked a2a: chunk 0 = tokens [0,1024), chunk 1 = [1024,2048)
            a2a_in = [
                [dramp.tile([NC, 128, 128], bf16, name=f"a2ai{u}_{c}") for c in range(2)]
                for u in range(B)
            ]
            a2a_out = [
                [dramp.tile([NC, 128, 128], bf16, name=f"a2ao{u}_{c}") for c in range(2)]
                for u in range(B)
            ]
            recip_dram = dramp.tile([B, 8, 512], f32)

            def collective(u, c):
                nc.gpsimd.collective_compute(
                    "AllToAll",
                    mybir.AluOpType.bypass,
                    replica_groups=[list(range(NC))],
                    ins=[a2a_in[u][c].opt()],
                    outs=[a2a_out[u][c].opt()],
                )

            def o_projection(u, c):
                g = op_.tile([128, 8, 128], bf16, tag="g", name="g")
                nc.gpsimd.dma_start(g[:], a2a_out[u][c].rearrange("s p c -> p s c"))
                y_ps = psp.tile([128, 1024], f32, tag="ps", name="y_ps")
                for os_ in range(2):
                    for dc in range(8):
                        nc.tensor.matmul(
                            y_ps[:, os_ * 512:(os_ + 1) * 512],
                            g[:, dc, :],
                            wo_sb[:, dc, os_ * 512:(os_ + 1) * 512],
                            start=(dc == 0), stop=(dc == 7),
                            skip_group_check=True,
                        )
                y_sb = yp.tile([128, D], f32, tag="y", name="y_sb")
                nc.vector.tensor_copy(out=y_sb[:], in_=y_ps[:])
                nc.sync.dma_start(y_d[u, c], y_sb[:])

            def load_xt(u, first):
                tiles = []
                for tt in range(4):
                    ts = slice(tt * 512, (tt + 1) * 512)
                    xt_sb = xtp.tile([128, 8, 512], bf16, tag="xt", name="xt_sb")
                    src = xt_d[u, :, ts].rearrange("(dc p) s -> p dc s", dc=8)
                    nc.sync.dma_start(xt_sb[:, 0:4, :], src[:, 0:4, :])
                    nc.sync.dma_start(xt_sb[:, 4:8, :], src[:, 4:8, :])
                    if first and tt == 0:
                        nc.sync.dma_start(
                            wk_sb[:], wkt_d[:, :].rearrange("(dc p) c -> p dc c", dc=8))
                        nc.sync.dma_start(
                            wv_sb[:], wvt_d[:, :].rearrange("(dc p) c -> p dc c", dc=8))
                        # non-critical consts after the first xt tile
                        nc.gpsimd.dma_start(cost[:, 0:1024], cost_d[:, 0:1024])
                        nc.gpsimd.dma_start(sinmt[:, 0:1024], sinmt_d[:, 0:1024])
                        nc.gpsimd.dma_start(ident[:], ident_d[:])
                        nc.gpsimd.dma_start(identb[:], identb_d[:])
                        nc.gpsimd.dma_start(ones_sb[:], ones_d[:])
                        nc.gpsimd.dma_start(trimask[:], trimask_d[:])
                        nc.gpsimd.dma_start(cost[:, 1024:2048], cost_d[:, 1024:2048])
                        nc.gpsimd.dma_start(sinmt[:, 1024:2048], sinmt_d[:, 1024:2048])
                    tiles.append(xt_sb)
                return tiles

            xts = {0: load_xt(0, True)}

            for u in range(B):
                # ================= projections + RoPE =================
                qT = qkp.tile([128, S], bf16, tag="qT", name="qT")
                kT = qkp.tile([128, S], bf16, tag="kT", name="kT")
                v_sb = vp.tile([128, 16, 130], f32r, tag="v", name="v_sb")

                for tt in range(4):
                    ts = slice(tt * 512, (tt + 1) * 512)
                    xt_sb = xts[u][tt]
                    qk_ps = psp.tile([128, 1024], f32, tag="ps", name="qk_ps")
                    v_ps = psp.tile([128, 1024], f32, tag="ps", name="v_ps")
                    for w_sb, dst in ((wq_sb, qk_ps[:, 0:512]),
                                      (wk_sb, qk_ps[:, 512:1024]),
                                      (wv_sb, v_ps[:, 0:512])):
                        for dc in range(8):
                            nc.tensor.matmul(dst, w_sb[:, dc, :], xt_sb[:, dc, :],
                                             start=(dc == 0), stop=(dc == 7))

                    # RoPE: dst = q*cos + pairswap(q)*sinm  (bf16 out)
                    for src_, dst in ((qk_ps[:, 0:512], qT), (qk_ps[:, 512:1024], kT)):
                        qs = rtmp.tile([128, 512], f32, tag="qs", name="qs")
                        t2 = rtmp.tile([128, 512], bf16, tag="t2", name="t2")
                        nc.vector.stream_shuffle(qs[:], src_, SWAP_MASK)
                        nc.vector.tensor_tensor(out=dst[:, ts], in0=src_, in1=cost[:, ts], op=MUL)
                        nc.vector.tensor_tensor(out=t2[:], in0=qs[:], in1=sinmt[:, ts], op=MUL)
                        nc.vector.tensor_tensor(out=dst[:, ts], in0=dst[:, ts], in1=t2[:], op=ADD)

                    # v -> token-major via PE transpose; ones columns appended
                    vtmp = rtmp.tile([128, 512], f32, tag="vtmp", name="vtmp")
                    nc.vector.tensor_copy(out=vtmp[:], in_=v_ps[:, 0:512])
                    for s4 in range(4):
                        kt = tt * 4 + s4
                        tr = v_ps[:, 512 + s4 * 128: 512 + (s4 + 1) * 128]
                        nc.tensor.transpose(tr, vtmp[:, s4 * 128:(s4 + 1) * 128], ident[:])
                        dst = v_sb[:, kt, :].rearrange("p (u c) -> p u c", u=2)[:, :, 0:64]
                        src_ = tr.rearrange("p (u c) -> p u c", u=2)
                        nc.vector.tensor_copy(out=dst, in_=src_)
                    nc.vector.tensor_copy(out=v_sb[:, tt * 4:(tt + 1) * 4, 64:65],
                                          in_=ones_sb[:, tt * 4:(tt + 1) * 4])
                    nc.vector.tensor_copy(out=v_sb[:, tt * 4:(tt + 1) * 4, 129:130],
                                          in_=ones_sb[:, tt * 4:(tt + 1) * 4])

                if u == 0:
                    # prefetch unit 1's activations during unit 0's attention
                    xts[1] = load_xt(1, False)

                # ================= attention =================
                for qi in range(4):
                    qsl = slice(qi * 512, (qi + 1) * 512)
                    outT = psp.tile([128, 1024], f32, tag="ps", name="outT")
                    n_kt = 4 * qi + 4
                    # software pipeline (depth 2): AV for tile kt is emitted
                    # after scores/exp of tile kt+2 so the PE isn't stalled
                    # on the ACT engine's exp
                    pend = []  # (at, kt, q0)

                    def flush_av(last):
                        at_, kt_, q0 = pend.pop(0)
                        for h in (0, 1):
                            nc.tensor.matmul(
                                outT[0:65, h * 512 + q0:(h + 1) * 512],
                                v_sb[:, kt_, :].rearrange("p (u c) -> p u c", u=2)[:, h, :],
                                at_[:, h * 512 + q0:(h + 1) * 512],
                                start=(kt_ == 0), stop=last,
                                skip_group_check=True,
                            )

                    for kt in range(n_kt):
                        ksl = slice(kt * 128, (kt + 1) * 128)
                        rel = kt - 4 * qi  # >=0 on the diagonal block
                        q0 = 128 * rel if rel > 0 else 0
                        sc = psp.tile([128, 1024], f32, tag="ps", name="sc")
                        at = attnp.tile([128, 1024], f32r, tag="at", name="at")
                        for h in (0, 1):
                            hp = slice(h * 64, (h + 1) * 64)
                            nc.tensor.matmul(
                                sc[:, h * 512 + q0:(h + 1) * 512],
                                kT[hp, ksl],
                                qT[hp, qsl][:, q0:512],
                                start=True, stop=(rel < 0),
                                skip_group_check=True,
                            )
                        if rel >= 0:
                            # causal triangle at q columns [q0, q0+128)
                            for h in (0, 1):
                                nc.tensor.matmul(
                                    sc[:, h * 512 + q0: h * 512 + q0 + 128],
                                    identb[:],
                                    trimask[:],
                                    start=False, stop=True,
                                    skip_group_check=True,
                                )
                        if len(pend) >= 2:
                            flush_av(False)
                        if q0 == 0:
                            nc.scalar.activation(out=at[:], in_=sc[:], func=EXP, scale=0.125)
                        else:
                            scs = sc.rearrange("p (h q) -> p h q", h=2)[:, :, q0:512]
                            ats = at.rearrange("p (h q) -> p h q", h=2)[:, :, q0:512]
                            nc.scalar.activation(out=ats, in_=scs, func=EXP, scale=0.125)
                        pend.append((at, kt, q0))
                    while len(pend) > 1:
                        flush_av(False)
                    flush_av(True)
                    if u == 1 and qi == 3:
                        # 16 matmuls for the PE while the DVE normalizes qi3;
                        # emitted before cc(1,1) exists so it only waits on A1
                        o_projection(1, 0)

                    # unload + normalize + ship this q-tile.  1/den per head is
                    # broadcast across 64 partitions by a K=2 selector matmul
                    # into the (now free) last score tile's PSUM region.
                    aoT = aop.tile([128, 512], bf16, tag="aoT", name="aoT")
                    rb = rbp.tile([128, 512], f32, tag="rb", name="rb")
                    for h in (0, 1):
                        nc.vector.tensor_copy(
                            out=aoT[h * 64:(h + 1) * 64, :],
                            in_=outT[0:64, h * 512:(h + 1) * 512],
                        )
                        dent = smallp.tile([1, 512], f32, tag=f"dent{h}",
                                           name="dent", bufs=2)
                        rc = smallp.tile([1, 512], f32, tag=f"rc{h}",
                                         name="rc", bufs=2)
                        nc.vector.tensor_copy(
                            out=dent[0:1, :], in_=outT[64:65, h * 512:(h + 1) * 512]
                        )
                        nc.vector.reciprocal_approx_fast(out=rc[0:1, :], in_=dent[0:1, :])
                        nc.sync.dma_start(
                            recip_dram[u, h * 4 + qi: h * 4 + qi + 1, :], rc[0:1, :]
                        )
                        nc.gpsimd.dma_start(
                            rb[h * 64:(h + 1) * 64, :],
                            recip_dram[u, h * 4 + qi: h * 4 + qi + 1, :].to_broadcast([64, 512]),
                        )
                    nc.vector.tensor_tensor(out=aoT[:], in0=aoT[:], in1=rb[:], op=MUL)
                    # ship: chunk c = qi//2, slots 4*(qi%2) .. +4
                    c = qi // 2
                    j0 = 4 * (qi % 2)
                    nc.sync.dma_start(
                        a2a_in[u][c][j0:j0 + 4].rearrange("s p c -> p s c"),
                        aoT[:].rearrange("p (s c) -> p s c", s=4),
                    )
                    if u == 0 and qi == 1:
                        collective(0, 0)
                    elif u == 0 and qi == 3:
                        collective(0, 1)
                        # O-projection weights: off the startup critical path
                        for dc in range(8):
                            nc.gpsimd.dma_start(
                                wo_sb[:, dc, :], wot_d[dc * 128:(dc + 1) * 128, :]
                            )
                    elif u == 1 and qi == 1:
                        o_projection(0, 0)
                        collective(1, 0)
                    elif u == 1 and qi == 2:
                        o_projection(0, 1)
                    elif u == 1 and qi == 3:
                        collective(1, 1)

            o_projection(1, 1)

    nc.compile()
    return nc


def _host_inputs(x, wq, wk, wv, wo):
    import ml_dtypes

    bf = ml_dtypes.bfloat16
    xt = np.ascontiguousarray(x.transpose(0, 2, 1)).astype(bf)
    wot = np.ascontiguousarray(wo.T).astype(bf)

    p = np.arange(128)
    invf = THETA ** (-2.0 * ((p % 64) // 2) / 64.0)
    ang = invf[:, None] * np.arange(S)[None, :]
    cost = np.cos(ang).astype(bf)
    sinmt = (np.sin(ang) * np.where(p % 2 == 0, -1.0, 1.0)[:, None]).astype(bf)

    i = np.arange(128)[:, None]
    j = np.arange(128)[None, :]
    # additive causal triangle: 0 where allowed (j >= i), -1e9 where masked
    trimask = np.where(j >= i, 0.0, -1e9).astype(bf)
    ident = np.eye(128, dtype=np.float32)

    in_maps = []
    for c in range(NC):
        sl = slice(c * 128, (c + 1) * 128)
        in_maps.append({
            "xt": xt,
            "wqt": np.ascontiguousarray(wq[sl, :].T).astype(bf),
            "wkt": np.ascontiguousarray(wk[sl, :].T).astype(bf),
            "wvt": np.ascontiguousarray(wv[sl, :].T).astype(bf),
            "wot": wot,
            "cost": cost,
            "sinmt": sinmt,
            "trimask": trimask,
            "ident": ident,
            "identb": ident.astype(bf),
            "ones": np.ones((128, 16), np.float32),
        })
    return in_maps


def kernel(x, wq, wk, wv, wo, _trace=False):
    from concourse.bass_utils import run_bass_kernel_spmd

    if "nc" not in _COMPILED:
        _COMPILED["nc"] = _build()
    nc = _COMPILED["nc"]

    in_maps = _host_inputs(
        np.asarray(x, np.float32), np.asarray(wq, np.float32),
        np.asarray(wk, np.float32), np.asarray(wv, np.float32),
        np.asarray(wo, np.float32),
    )
    res = run_bass_kernel_spmd(nc, in_maps, core_ids=list(range(NC)), trace=_trace)
    _COMPILED["last_result"] = res

    y = np.zeros((B, S, D), np.float32)
    for c in range(NC):
        yc = res.results[c]["y"]  # [B, 2, 128, D]
        for u in range(B):
            y[u, 128 * c: 128 * c + 128, :] = yc[u, 0]
            y[u, 1024 + 128 * c: 1024 + 128 * c + 128, :] = yc[u, 1]
    return y
